# revision 1
# baseline (speedup 1.0000x reference)
# GAT 2-layer kernel for trn2 x8 — host prep + bass program + runner.
# Sharding: dst-node partition (graph parallel). Each core owns a contiguous
# 6272-node block and processes all edges into it; layer-1 node features are
# computed replicated; layer-2 node tables are exchanged via AllGather.
from contextlib import ExitStack

import numpy as np

import concourse.bass as bass
import concourse.bacc as bacc
import concourse.tile as tile
from concourse import mybir, library_config
from concourse.bass_utils import run_bass_kernel_spmd
from concourse.masks import make_identity

# ---- problem constants ----
N = 50000
DIN = 128
H1, C1 = 4, 32
C2 = 32
NCORE = 8

NODE_PAD = 50048          # 391 * 128
NT = NODE_PAD // 128      # 391 node tiles
CS = 6272                 # slots per core (49*128); node n -> core n//CS, rank n%CS
NMACRO = CS // 128        # 49
WIN = 32                  # dst-window width (psum base must be 32-aligned)
NW = 128 // WIN
EPT = 128                 # max edges per tile
GM = 3                    # macros per gather group

F1 = 132                  # layer-1 msg width: 4 head blocks of [32 feats | 1.0]
ROW1_H = 148              # f16 cols per table1 row (296 B)
ROW1_W = 74               # f32 cols
F2 = 33                   # layer-2 msg width: 32 feats + 1.0
ROW2_H = 40               # f16 cols per table2 row (80 B)
ROW2_W = 20
SHIFT1 = 6.0
SHIFT2 = 2.0

FP16 = mybir.dt.float16
FP32 = mybir.dt.float32
I32 = mybir.dt.int32
I16 = mybir.dt.int16

HA = 25088                # xT gather first-half rows
HB = NODE_PAD - HA        # 24960


def _phys1(n):
    return (n % 128) * NT + (n // 128)


def _phys2(core, slot):
    return core * CS + (slot % 128) * NMACRO + (slot // 128)


def _wrap16(a):
    n = len(a)
    assert n % 16 == 0
    out = np.empty((16, n // 16), np.int16)
    for k in range(n):
        out[k % 16, k // 16] = a[k]
    return out


def host_prep(inputs):
    """Canonical tile schedule shared by all cores + per-core blobs."""
    ei = np.asarray(inputs["edge_index"])
    src = np.concatenate([ei[0], np.arange(N, dtype=np.int64)]).astype(np.int64)
    dst = np.concatenate([ei[1], np.arange(N, dtype=np.int64)]).astype(np.int64)

    cores = []
    for c in range(NCORE):
        base = CS * c
        hi = min(base + CS, N)
        m = (dst >= base) & (dst < hi)
        s_c = src[m]
        r_c = (dst[m] - base).astype(np.int64)
        order = np.argsort(r_c, kind="stable")
        s_c = s_c[order]
        r_c = r_c[order]
        deg = np.bincount(r_c, minlength=CS)
        assert deg.max() <= EPT
        starts = np.zeros(CS + 1, np.int64)
        np.cumsum(deg, out=starts[1:])
        wt = {}
        for mac in range(NMACRO):
            for w in range(NW):
                lo = mac * 128 + w * WIN
                tl = []
                cur, cur_e = [], 0
                for r in range(lo, lo + WIN):
                    dd = int(deg[r])
                    if dd == 0:
                        continue
                    if cur_e + dd > EPT:
                        tl.append(cur)
                        cur, cur_e = [], 0
                    cur.append(r)
                    cur_e += dd
                if cur:
                    tl.append(cur)
                wt[(mac, w)] = tl
        cores.append(dict(s=s_c, starts=starts, wt=wt))

    tpw = np.zeros((NMACRO, NW), np.int32)
    for cc in cores:
        for (mac, w), tl in cc["wt"].items():
            tpw[mac, w] = max(tpw[mac, w], len(tl))
    sched = []  # (macro, window, first_of_window, last_of_window)
    for mac in range(NMACRO):
        for w in range(NW):
            nt = int(tpw[mac, w])
            assert nt >= 1
            for j in range(nt):
                sched.append((mac, w, j == 0, j == nt - 1))
    T = len(sched)

    per_core = []
    for c, cc in enumerate(cores):
        pat = np.zeros((128, T * WIN), np.float16)
        idx_s1 = np.zeros((128, T), np.int32)
        idx_d1 = np.zeros((128, T), np.int32)
        idx_s2 = np.zeros((128, T), np.int32)
        idx_d2 = np.zeros((128, T), np.int32)
        starts, s_c = cc["starts"], cc["s"]
        t = 0
        for mac in range(NMACRO):
            for w in range(NW):
                tl = cc["wt"][(mac, w)]
                for j in range(int(tpw[mac, w])):
                    if j < len(tl):
                        e = 0
                        for r in tl[j]:
                            for k in range(starts[r], starts[r + 1]):
                                sn = int(s_c[k])
                                pat[e, t * WIN + (r - mac * 128 - w * WIN)] = 1.0
                                idx_s1[e, t] = _phys1(sn)
                                idx_d1[e, t] = _phys1(CS * c + r)
                                idx_s2[e, t] = _phys2(sn // CS, sn % CS)
                                idx_d2[e, t] = _phys2(c, r)
                                e += 1
                    t += 1
        per_core.append(dict(pat=pat, idx_s1=idx_s1, idx_d1=idx_d1,
                             idx_s2=idx_s2, idx_d2=idx_d2))
    return sched, per_core


def make_in_maps(inputs, sched, per_core):
    x = np.asarray(inputs["x"], np.float32)
    xpad = np.zeros((NODE_PAD, DIN), np.float32)
    xpad[:N] = x
    W1 = np.asarray(inputs["W1"], np.float32)
    perm1 = np.zeros(128, np.int64)
    for cc_ in range(C1):
        for h in range(H1):
            perm1[cc_ * H1 + h] = h * C1 + cc_
    W1p = np.ascontiguousarray(W1[:, perm1])
    a_s1 = np.asarray(inputs["att_src1"], np.float32)
    a_d1 = np.asarray(inputs["att_dst1"], np.float32)
    attv1 = np.zeros((128, 8), np.float32)
    for h in range(H1):
        for cc_ in range(C1):
            attv1[cc_ * H1 + h, h] = a_s1[h, cc_]
            attv1[cc_ * H1 + h, 4 + h] = a_d1[h, cc_]
    W2 = np.asarray(inputs["W2"], np.float32)
    W2p = np.ascontiguousarray(W2[perm1, :])
    a_s2 = np.asarray(inputs["att_src2"], np.float32)[0]
    a_d2 = np.asarray(inputs["att_dst2"], np.float32)[0]
    attv2 = np.zeros((C2, 2), np.float32)
    attv2[:, 0] = a_s2
    attv2[:, 1] = a_d2
    assert not np.any(np.asarray(inputs["bias1"])) and \
        not np.any(np.asarray(inputs["bias2"])), "nonzero bias unsupported"

    in_maps = []
    for c in range(NCORE):
        pc = per_core[c]
        in_maps.append(dict(
            xpad=xpad, W1p=W1p, attv1=attv1, W2p=W2p, attv2=attv2,
            pat=pc["pat"], idx_s1=pc["idx_s1"], idx_d1=pc["idx_d1"],
            idx_s2=pc["idx_s2"], idx_d2=pc["idx_d2"],
        ))
    return in_maps


def build_program(sched, debug=False):
    T = len(sched)
    nc = bacc.Bacc("TRN2", target_bir_lowering=False, debug=False, num_devices=NCORE)
    tc = tile.TileContext(nc)

    xpad_d = nc.dram_tensor("xpad", [NODE_PAD, DIN], FP32, kind="ExternalInput")
    W1p_d = nc.dram_tensor("W1p", [128, 128], FP32, kind="ExternalInput")
    attv1_d = nc.dram_tensor("attv1", [128, 8], FP32, kind="ExternalInput")
    W2p_d = nc.dram_tensor("W2p", [128, C2], FP32, kind="ExternalInput")
    attv2_d = nc.dram_tensor("attv2", [C2, 2], FP32, kind="ExternalInput")
    pat_d = nc.dram_tensor("pat", [128, T * WIN], FP16, kind="ExternalInput")
    idx_s1_d = nc.dram_tensor("idx_s1", [128, T], I32, kind="ExternalInput")
    idx_d1_d = nc.dram_tensor("idx_d1", [128, T], I32, kind="ExternalInput")
    idx_s2_d = nc.dram_tensor("idx_s2", [128, T], I32, kind="ExternalInput")
    idx_d2_d = nc.dram_tensor("idx_d2", [128, T], I32, kind="ExternalInput")
    out2_d = nc.dram_tensor("out2", [CS, C2], FP32, kind="ExternalOutput")

    tab1_d = nc.dram_tensor("tab1i", [NODE_PAD, ROW1_H], FP16)
    if debug:
        dbg_tab1 = nc.dram_tensor("dbg_tab1", [256, ROW1_H], FP16, kind="ExternalOutput")
        dbg_g1 = nc.dram_tensor("dbg_g1", [128, 280], FP16, kind="ExternalOutput")
        dbg_u = nc.dram_tensor("dbg_u", [128, 8], FP32, kind="ExternalOutput")
        dbg_msg = nc.dram_tensor("dbg_msg", [128, 2 * F1], FP16, kind="ExternalOutput")
        dbg_helu = nc.dram_tensor("dbg_helu", [128, 128], FP16, kind="ExternalOutput")
        dbg_ht = nc.dram_tensor("dbg_ht", [128, 128], FP16, kind="ExternalOutput")
        dbg_tb2g = nc.dram_tensor("dbg_tb2g", [256, ROW2_H], FP16, kind="ExternalOutput")
        dbg_agg = nc.dram_tensor("dbg_agg", [128, F1], FP32, kind="ExternalOutput")
        dbg_g2 = nc.dram_tensor("dbg_g2", [128, 2 * ROW2_H], FP16, kind="ExternalOutput")
        dbg_u2 = nc.dram_tensor("dbg_u2", [128, 8], FP32, kind="ExternalOutput")
        dbg_m2 = nc.dram_tensor("dbg_m2", [128, 2 * 34], FP16, kind="ExternalOutput")
        dbg_ag2 = nc.dram_tensor("dbg_ag2", [128, F2], FP32, kind="ExternalOutput")
        dbg_ad2 = nc.dram_tensor("dbg_ad2", [128, 4], FP32, kind="ExternalOutput")
    tb2l_d = nc.dram_tensor("tb2li", [CS, ROW2_H], FP16)
    tb2g_d = nc.dram_tensor("tb2gi", [NCORE * CS, ROW2_H], FP16, addr_space="Shared")

    # group boundaries (GM macros per gather group)
    mstart = [0] * (NMACRO + 1)
    for i, (mac, w, fw, lw) in enumerate(sched):
        mstart[mac + 1] = i + 1
    groups = []
    for g0 in range(0, NMACRO, GM):
        g1_ = min(g0 + GM, NMACRO)
        groups.append((mstart[g0], mstart[g1_], g0, g1_))
    GT_MAX = max(t1 - t0 for t0, t1, _, _ in groups)

    with tc, ExitStack() as ctx:
        cc_sem = ctx.enter_context(nc.semaphore("ccsem"))
        const = ctx.enter_context(tc.tile_pool(name="const", bufs=1))

        w1_sb = const.tile([128, 128], FP32)
        nc.sync.dma_start(w1_sb[:], W1p_d.ap())
        attv1_sb = const.tile([128, 8], FP32)
        nc.sync.dma_start(attv1_sb[:], attv1_d.ap())
        w2_sb = const.tile([128, C2], FP32)
        nc.sync.dma_start(w2_sb[:], W2p_d.ap())
        attv2_sb = const.tile([C2, 2], FP32)
        nc.sync.dma_start(attv2_sb[:], attv2_d.ap())

        ident = const.tile([128, 128], FP32)
        make_identity(nc, ident[:])
        ident16 = const.tile([128, 128], FP16)
        make_identity(nc, ident16[:])
        nsh1 = const.tile([128, 1], FP32)
        nc.gpsimd.memset(nsh1[:], -SHIFT1)
        nsh2 = const.tile([128, 1], FP32)
        nc.gpsimd.memset(nsh2[:], -SHIFT2)

        with tc.tile_pool(name="ps0", bufs=1, space="PSUM") as ps0:
            w1t_ps = ps0.tile([128, 128], FP32, space="PSUM")
            nc.tensor.transpose(w1t_ps[:], w1_sb[:], ident[:])
            w1t_sb = const.tile([128, 128], FP32)
            nc.vector.tensor_copy(w1t_sb[:], w1t_ps[:])
            wat1_ps = ps0.tile([128, 8], FP32, space="PSUM")
            nc.tensor.matmul(wat1_ps[:], w1t_sb[:], attv1_sb[:], start=True, stop=True)
            rhs1 = const.tile([128, 136], FP16)
            nc.vector.tensor_copy(rhs1[:, 0:128], w1_sb[:])
            nc.vector.tensor_copy(rhs1[:, 128:136], wat1_ps[:])
            w2t_ps = ps0.tile([C2, 128], FP32, space="PSUM")
            nc.tensor.transpose(w2t_ps[:], w2_sb[:], ident[:])
            w2t_sb = const.tile([C2, 128], FP32)
            nc.vector.tensor_copy(w2t_sb[:], w2t_ps[:])
            wat2_ps = ps0.tile([128, 2], FP32, space="PSUM")
            nc.tensor.matmul(wat2_ps[:], w2t_sb[:], attv2_sb[:], start=True, stop=True)
            rhs2 = const.tile([128, C2 + 2], FP16)
            nc.vector.tensor_copy(rhs2[:, 0:C2], w2_sb[:])
            nc.vector.tensor_copy(rhs2[:, C2:C2 + 2], wat2_ps[:])

        # ---------------- phase A: xT (PE transpose) + table1 ----------------
        xpad_v = xpad_d.ap().rearrange("(t p) c -> p t c", p=128)  # [128, NT, 128]
        with tc.tile_pool(name="xc", bufs=2) as xcp, \
             tc.tile_pool(name="xt", bufs=4) as xtp, \
             tc.tile_pool(name="pa_ps", bufs=3, space="PSUM") as paps, \
             tc.tile_pool(name="pa_st", bufs=3) as past:
            CH = 4
            for nt0 in range(0, NT, CH):
                nch = min(CH, NT - nt0)
                xc = xcp.tile([128, CH * 128], FP32, tag="xc")
                nc.sync.dma_start(
                    xc[:, 0:nch * 128].rearrange("p (t c) -> p t c", c=128),
                    xpad_v[:, nt0:nt0 + nch, :])
                st = past.tile([128, CH * ROW1_H], FP16, tag="stage")
                stf = st[:].bitcast(FP32)
                nc.vector.memset(
                    st[:].rearrange("p (t c) -> p t c", c=ROW1_H)[:, 0:nch, 128:132],
                    1.0)
                for k in range(nch):
                    xtps = paps.tile([128, 128], FP32, space="PSUM", tag="xtp")
                    nc.tensor.transpose(xtps[:], xc[:, k * 128:(k + 1) * 128],
                                        ident[:])
                    xts = xtp.tile([128, 128], FP16, tag="xts")
                    nc.scalar.activation(xts[:], xtps[:],
                                         mybir.ActivationFunctionType.Copy)
                    h1ps = paps.tile([128, 136], FP32, space="PSUM", tag="h1")
                    nc.tensor.matmul(h1ps[:], xts[:], rhs1[:], start=True, stop=True)
                    nc.scalar.activation(
                        st[:, k * ROW1_H: k * ROW1_H + 128], h1ps[:, 0:128],
                        mybir.ActivationFunctionType.Copy)
                    nc.vector.tensor_copy(
                        stf[:, k * ROW1_W + 66: k * ROW1_W + 74], h1ps[:, 128:136])
                dst_ap = tab1_d.ap().rearrange("(p t) c -> p t c", p=128)
                nc.sync.dma_start(
                    dst_ap[:, nt0:nt0 + nch, :],
                    st[:].rearrange("p (t c) -> p t c", c=ROW1_H)[:, 0:nch, :])

        tc.strict_bb_all_engine_barrier()

        # ---------------- resident edge data ----------------
        epool = ctx.enter_context(tc.tile_pool(name="edata", bufs=1))
        pat_sb = epool.tile([128, T * WIN], FP16)
        nc.sync.dma_start(pat_sb[:], pat_d.ap())
        ixs1 = epool.tile([128, T], I32)
        nc.sync.dma_start(ixs1[:], idx_s1_d.ap())
        ixd1 = epool.tile([128, T], I32)
        nc.sync.dma_start(ixd1[:], idx_d1_d.ap())
        ixs2 = epool.tile([128, T], I32)
        nc.sync.dma_start(ixs2[:], idx_s2_d.ap())
        ixd2 = epool.tile([128, T], I32)
        nc.sync.dma_start(ixd2[:], idx_d2_d.ap())
        htpool = ctx.enter_context(tc.tile_pool(name="ht", bufs=1))
        ht = htpool.tile([128, CS], FP16)

        # ---------------- phase B: layer-1 edges ----------------
        tab1_h = tab1_d.ap()
        tab1_f = tab1_d.ap().bitcast(FP32)
        with tc.tile_pool(name="g1", bufs=2) as g1p, \
             tc.tile_pool(name="zu", bufs=2) as zup, \
             tc.tile_pool(name="msgp", bufs=2) as msgp, \
             tc.tile_pool(name="aggp", bufs=4, space="PSUM") as aggp, \
             tc.tile_pool(name="htps", bufs=2, space="PSUM") as htpsp, \
             tc.tile_pool(name="hn", bufs=2) as hnp:
            for (t0, t1, m0, m1) in groups:
                gt = t1 - t0
                g1 = g1p.tile([128, GT_MAX * 140], FP16, tag="g1")
                adg = zup.tile([128, GT_MAX * 4], FP32, tag="ad")
                for t in range(t0, t1):
                    j = t - t0
                    nc.gpsimd.indirect_dma_start(
                        out=g1[:, j * 140:(j + 1) * 140], out_offset=None, in_=tab1_h,
                        in_offset=bass.IndirectOffsetOnAxis(ap=ixs1[:, t:t + 1], axis=0))
                    nc.gpsimd.indirect_dma_start(
                        out=adg[:, j * 4:(j + 1) * 4], out_offset=None, in_=tab1_f,
                        in_offset=bass.IndirectOffsetOnAxis(ap=ixd1[:, t:t + 1], axis=0),
                        element_offset=70)
                g1f = g1[:].bitcast(FP32)
                z = zup.tile([128, GT_MAX * 4], FP32, tag="z")
                nc.vector.tensor_tensor(
                    out=z[:, 0:gt * 4].rearrange("p (t h) -> p t h", h=4),
                    in0=g1f[:, 0:gt * 70].rearrange("p (t c) -> p t c", c=70)[:, :, 66:70],
                    in1=adg[:, 0:gt * 4].rearrange("p (t h) -> p t h", h=4),
                    op=mybir.AluOpType.add)
                zs = zup.tile([128, GT_MAX * 4], FP32, tag="zs")
                nc.vector.tensor_scalar_mul(zs[:, 0:gt * 4], z[:, 0:gt * 4], 0.2)
                nc.vector.tensor_tensor(out=z[:, 0:gt * 4], in0=z[:, 0:gt * 4],
                                        in1=zs[:, 0:gt * 4], op=mybir.AluOpType.max)
                u = zup.tile([128, GT_MAX * 4], FP16, tag="u")
                nc.scalar.activation(u[:, 0:gt * 4], z[:, 0:gt * 4],
                                     mybir.ActivationFunctionType.Exp, bias=nsh1[:])
                msg = msgp.tile([128, GT_MAX * F1], FP16, tag="msg")
                nc.vector.tensor_tensor(
                    out=msg[:, 0:gt * F1].rearrange("p (t c h) -> p t c h", c=33, h=4),
                    in0=g1[:, 0:gt * 140].rearrange("p (t c) -> p t c", c=140)[:, :, 0:132]
                        .rearrange("p t (c h) -> p t c h", h=4),
                    in1=u[:, 0:gt * 4].rearrange("p (t h) -> p t h", h=4)
                        .unsqueeze(2).broadcast_to([128, gt, 33, 4]),
                    op=mybir.AluOpType.mult)
                if debug and t0 == 0:
                    nc.sync.dma_start(dbg_g1.ap(), g1[:, 0:280])
                    udbg = zup.tile([128, 8], FP32, tag="udbg")
                    nc.vector.tensor_copy(udbg[:], u[:, 0:8])
                    nc.sync.dma_start(dbg_u.ap(), udbg[:])
                    nc.sync.dma_start(dbg_msg.ap(), msg[:, 0:2 * F1])
                for mac in range(m0, m1):
                    agg = aggp.tile([128, F1], FP32, space="PSUM", tag="agg")
                    for t in range(mstart[mac], mstart[mac + 1]):
                        _, w, fw, lw = sched[t]
                        nc.tensor.matmul(
                            agg[w * WIN:(w + 1) * WIN, :],
                            pat_sb[:, t * WIN:(t + 1) * WIN],
                            msg[:, (t - t0) * F1:(t - t0 + 1) * F1],
                            start=fw, stop=lw, tile_position=(0, w * WIN))
                    if debug and mac == 0:
                        aggd = hnp.tile([128, F1], FP32, tag="aggd")
                        nc.vector.tensor_copy(aggd[:], agg[:])
                        nc.sync.dma_start(dbg_agg.ap(), aggd[:])
                    r = hnp.tile([128, 4], FP32, tag="r")
                    nc.vector.reciprocal(r[:], agg[:, 128:132])
                    hn = hnp.tile([128, 128], FP16, tag="hn")
                    nc.vector.tensor_tensor(
                        out=hn[:].rearrange("p (c h) -> p c h", h=4),
                        in0=agg[:, 0:128].rearrange("p (c h) -> p c h", h=4),
                        in1=r[:].unsqueeze(1).broadcast_to([128, 32, 4]),
                        op=mybir.AluOpType.mult)
                    a = hnp.tile([128, 128], FP16, tag="elua")
                    nc.vector.tensor_scalar_min(a[:], hn[:], 0.0)
                    e = hnp.tile([128, 128], FP16, tag="elue")
                    nc.scalar.activation(e[:], a[:], mybir.ActivationFunctionType.Exp)
                    em1 = hnp.tile([128, 128], FP16, tag="eluem")
                    nc.vector.tensor_scalar_add(em1[:], e[:], -1.0)
                    helu = hnp.tile([128, 128], FP16, tag="helu")
                    nc.vector.tensor_tensor(out=helu[:], in0=hn[:], in1=em1[:],
                                            op=mybir.AluOpType.max)
                    htps = htpsp.tile([128, 128], FP16, space="PSUM", tag="htp")
                    nc.tensor.transpose(htps[:], helu[:], ident16[:])
                    nc.scalar.activation(
                        ht[:, mac * 128:(mac + 1) * 128], htps[:],
                        mybir.ActivationFunctionType.Copy)
                    if debug and mac == 0:
                        nc.sync.dma_start(dbg_helu.ap(), helu[:])

        tc.strict_bb_all_engine_barrier()

        # ---------------- phase C: q / table2 + AllGather ----------------
        with tc.tile_pool(name="pc_ps", bufs=6, space="PSUM") as pcps, \
             tc.tile_pool(name="pc_st", bufs=3) as pcst:
            CH2 = 8
            for sc0 in range(0, NMACRO, CH2):
                nch = min(CH2, NMACRO - sc0)
                st2 = pcst.tile([128, CH2 * ROW2_H], FP16, tag="st2")
                st2f = st2[:].bitcast(FP32)
                nc.vector.memset(
                    st2[:].rearrange("p (t c) -> p t c", c=ROW2_H)[:, 0:nch, C2:C2 + 1],
                    1.0)
                for k in range(nch):
                    scc = sc0 + k
                    qps = pcps.tile([128, C2 + 2], FP32, space="PSUM", tag="q")
                    nc.tensor.matmul(qps[:], ht[:, scc * 128:(scc + 1) * 128],
                                     rhs2[:], start=True, stop=True)
                    nc.scalar.activation(
                        st2[:, k * ROW2_H: k * ROW2_H + C2], qps[:, 0:C2],
                        mybir.ActivationFunctionType.Copy)
                    nc.vector.tensor_copy(
                        st2f[:, k * ROW2_W + 17: k * ROW2_W + 19], qps[:, C2:C2 + 2])
                dst2 = tb2l_d.ap().rearrange("(p t) c -> p t c", p=128)
                nc.sync.dma_start(
                    dst2[:, sc0:sc0 + nch, :],
                    st2[:].rearrange("p (t c) -> p t c", c=ROW2_H)[:, 0:nch, :])

        if debug:
            nc.sync.dma_start(dbg_tab1.ap(), tab1_d.ap()[0:256, :])
            nc.sync.dma_start(dbg_ht.ap(), ht[:, 0:128])
        tc.strict_bb_all_engine_barrier()
        with tc.tile_critical():
            nc.gpsimd.collective_compute(
                "AllGather", mybir.AluOpType.bypass,
                replica_groups=[list(range(NCORE))],
                ins=[tb2l_d.ap().opt()],
                outs=[tb2g_d.ap().opt()],
            ).then_inc(cc_sem)
            nc.gpsimd.wait_ge(cc_sem, 1)
        tc.strict_bb_all_engine_barrier()

        if debug:
            nc.sync.dma_start(dbg_tb2g.ap(), tb2g_d.ap()[CS:CS + 256, :])
        # ---------------- phase D: layer-2 edges ----------------
        tab2_h = tb2g_d.ap()
        tab2_f = tb2g_d.ap().bitcast(FP32)
        with tc.tile_pool(name="g2", bufs=2) as g2p, \
             tc.tile_pool(name="zu2", bufs=2) as zup2, \
             tc.tile_pool(name="msg2", bufs=2) as msgp2, \
             tc.tile_pool(name="agg2", bufs=6, space="PSUM") as aggp2, \
             tc.tile_pool(name="o2", bufs=2) as o2p:
            for (t0, t1, m0, m1) in groups:
                gt = t1 - t0
                g2 = g2p.tile([128, GT_MAX * ROW2_H], FP16, tag="g2")
                ad2 = zup2.tile([128, GT_MAX], FP32, tag="ad2")
                for t in range(t0, t1):
                    j = t - t0
                    nc.gpsimd.indirect_dma_start(
                        out=g2[:, j * ROW2_H:(j + 1) * ROW2_H], out_offset=None,
                        in_=tab2_h,
                        in_offset=bass.IndirectOffsetOnAxis(ap=ixs2[:, t:t + 1], axis=0))
                    nc.gpsimd.indirect_dma_start(
                        out=ad2[:, j:j + 1], out_offset=None, in_=tab2_f,
                        in_offset=bass.IndirectOffsetOnAxis(ap=ixd2[:, t:t + 1], axis=0),
                        element_offset=18)
                g2f = g2[:].bitcast(FP32)
                z2 = zup2.tile([128, GT_MAX], FP32, tag="z2")
                nc.vector.tensor_tensor(
                    out=z2[:, 0:gt].unsqueeze(2),
                    in0=g2f[:, 0:gt * 20].rearrange("p (t c) -> p t c", c=20)[:, :, 17:18],
                    in1=ad2[:, 0:gt].unsqueeze(2),
                    op=mybir.AluOpType.add)
                zs2 = zup2.tile([128, GT_MAX], FP32, tag="zs2")
                nc.vector.tensor_scalar_mul(zs2[:, 0:gt], z2[:, 0:gt], 0.2)
                nc.vector.tensor_tensor(out=z2[:, 0:gt], in0=z2[:, 0:gt],
                                        in1=zs2[:, 0:gt], op=mybir.AluOpType.max)
                u2 = zup2.tile([128, GT_MAX * 2], FP16, tag="u2")
                u2v = u2[:].rearrange("p (t j) -> p t j", j=2)
                nc.scalar.activation(u2v[:, 0:gt, 0:1], z2[:, 0:gt].unsqueeze(2),
                                     mybir.ActivationFunctionType.Exp, bias=nsh2[:])
                nc.vector.tensor_copy(u2v[:, 0:gt, 1:2], u2v[:, 0:gt, 0:1])
                msg2 = msgp2.tile([128, GT_MAX * 34], FP16, tag="m2")
                nc.vector.tensor_tensor(
                    out=msg2[:, 0:gt * 34].rearrange("p (t c j) -> p t c j", c=17, j=2),
                    in0=g2[:, 0:gt * ROW2_H].rearrange("p (t c) -> p t c", c=ROW2_H)
                        [:, :, 0:34].rearrange("p t (c j) -> p t c j", j=2),
                    in1=u2v[:, 0:gt, :].unsqueeze(2).broadcast_to([128, gt, 17, 2]),
                    op=mybir.AluOpType.mult)
                if debug and t0 == 0:
                    nc.sync.dma_start(dbg_g2.ap(), g2[:, 0:2 * ROW2_H])
                    u2d = zup2.tile([128, 8], FP32, tag="u2d")
                    nc.vector.tensor_copy(u2d[:], u2[:, 0:8])
                    nc.sync.dma_start(dbg_u2.ap(), u2d[:])
                    nc.sync.dma_start(dbg_m2.ap(), msg2[:, 0:2 * 34])
                    ad2d = zup2.tile([128, 4], FP32, tag="ad2d")
                    nc.vector.tensor_copy(ad2d[:], ad2[:, 0:4])
                    nc.sync.dma_start(dbg_ad2.ap(), ad2d[:])
                for mac in range(m0, m1):
                    agg2 = aggp2.tile([128, F2], FP32, space="PSUM", tag="ag2")
                    for t in range(mstart[mac], mstart[mac + 1]):
                        _, w, fw, lw = sched[t]
                        nc.tensor.matmul(
                            agg2[w * WIN:(w + 1) * WIN, :],
                            pat_sb[:, t * WIN:(t + 1) * WIN],
                            msg2[:, (t - t0) * 34:(t - t0) * 34 + F2],
                            start=fw, stop=lw, tile_position=(0, w * WIN))
                    if debug and mac == 0:
                        ag2d = o2p.tile([128, F2], FP32, tag="ag2d")
                        nc.vector.tensor_copy(ag2d[:], agg2[:])
                        nc.sync.dma_start(dbg_ag2.ap(), ag2d[:])
                    r2 = o2p.tile([128, 1], FP32, tag="r2")
                    nc.vector.reciprocal(r2[:], agg2[:, C2:C2 + 1])
                    o2 = o2p.tile([128, C2], FP32, tag="o2")
                    nc.vector.tensor_tensor(
                        out=o2[:], in0=agg2[:, 0:C2],
                        in1=r2[:].broadcast_to([128, C2]),
                        op=mybir.AluOpType.mult)
                    nc.sync.dma_start(out2_d.ap()[mac * 128:(mac + 1) * 128, :], o2[:])

    nc.compile()
    return nc


_CACHE = {}


def run(inputs, trace=False, debug=False):
    sched, per_core = host_prep(inputs)
    in_maps = make_in_maps(inputs, sched, per_core)
    key = (len(sched), debug)
    if key not in _CACHE:
        _CACHE[key] = build_program(sched, debug=debug)
    nc = _CACHE[key]
    res = run_bass_kernel_spmd(nc, in_maps, core_ids=list(range(NCORE)), trace=trace)
    outs = [r["out2"] for r in res.results]
    out = np.zeros((N, C2), np.float32)
    for c in range(NCORE):
        lo = c * CS
        hi = min(lo + CS, N)
        out[lo:hi] = outs[c][: hi - lo]
    return out, res


def kernel(**inputs):
    """Full-input GAT kernel: shards across 8 NeuronCores internally."""
    import numpy as _np
    out, _res = run(inputs)
    return out.astype(_np.float32)



# revision 5
# speedup vs baseline: 1.2957x; 1.2957x over previous
# GAT 2-layer kernel for trn2 x8 — host prep + bass program + runner.
# Sharding: dst-node partition (graph parallel). Each core owns a contiguous
# 6272-node block and processes all edges into it; layer-1 node features are
# computed replicated; layer-2 node tables are exchanged via AllGather.
#
# Edge gathers use the InstDMAGatherAnt primitive (one instruction per
# ~8k-edge group instead of per 128-edge tile) so GpSimd descriptor
# generation stays off the critical path. Tables larger than the int16
# index range use a sign-wrap trick: indices are uint16 bit patterns,
# the gather base sits at +32768 rows, and rows are stored at
# physical_row XOR 32768. Each group's index block ends with one all-zero
# dummy tile so the ucode's trailing-negative-index trim never fires.
from contextlib import ExitStack

import numpy as np

import concourse.bass as bass
import concourse.bacc as bacc
import concourse.tile as tile
from concourse import mybir
from concourse.bass_utils import run_bass_kernel_spmd
from concourse.masks import make_identity

# ---- problem constants ----
N = 50000
DIN = 128
H1, C1 = 4, 32
C2 = 32
NCORE = 8

NODE_PAD = 50048          # 391 * 128
NT = NODE_PAD // 128      # 391 node tiles
TPAD = 512                # padded tiles-per-partition in tab1 row index
CS = 6272                 # slots per core (49*128); node n -> core n//CS
NMACRO = CS // 128        # 49
WIN = 32                  # dst-window width (psum base must be 32-aligned)
NW = 128 // WIN
EPT = 128                 # max edges per tile
GM = 3                    # macros per gather group

ROW1 = 136                # tab1 row: [h (c,h)-interleaved 128 | a_src 4 | a_dst 4]
R1STEP = 256              # tab1 row stride in fp16 elems (512 B)
F1 = 132                  # gathered src row / msg width: [h 128 | a_src 4]
ROW2 = 34                 # tab2 row: [q 32 | a_src | a_dst]
ROW2S = 33                # gathered layer-2 src row: [q 32 | a_src]
R2STEP = 128              # tab2 row stride in fp16 elems (256 B)
F2 = 33                   # layer-2 msg width: 32 feats + u
SHIFT1 = 6.0
SHIFT2 = 2.0

TB1_ROWS = 65536          # tab1 rows (stored at phys1 XOR 32768)
TB2_ROWS = 65536          # tab2 rows (stored at g2 XOR 32768); g2 in [0, 50176)

FP16 = mybir.dt.float16
FP32 = mybir.dt.float32
I32 = mybir.dt.int32
I16 = mybir.dt.int16


def _phys1(n):
    return (n % 128) * TPAD + (n // 128)


def _g2(n):
    c, s = n // CS, n % CS
    return c * CS + (s % 128) * NMACRO + (s // 128)


def dma_gather_raw(g, out_ap, in_ap, idxs_ap, num_idxs, elem_size, elem_step,
                   queue_num=0):
    """bass.dma_gather without the elem_size%256 assert (non-transpose)."""
    from concourse import ap_utils
    from concourse.bass import exact_div
    assert idxs_ap.dtype == mybir.dt.int16
    assert in_ap.dtype == out_ap.dtype
    assert ap_utils.ap_is_contiguous(out_ap.ap[1:])
    assert ap_utils.ap_is_contiguous(idxs_ap.ap[1:])
    assert in_ap.ap[-1][1] == out_ap.ap[-1][1] == elem_size
    assert in_ap.ap[0][0] == elem_step
    stride_bytes = elem_step * mybir.dt.size(in_ap.dtype)
    stride_bytes_256 = exact_div(stride_bytes, 256)
    _in_ap = g.lower_ap_dma(in_ap, for_custom_bir_dma=True)
    _idxs_ap = g.lower_ap(idxs_ap)
    _out_ap = g.lower_ap(out_ap)
    return g.add_instruction(
        mybir.InstDMAGatherAnt(
            name=g.bass.get_next_instruction_name(),
            ins=[*_in_ap, _idxs_ap, g.lower_val_access(g.to_reg(num_idxs))],
            outs=[_out_ap],
            transpose=False,
            num_idxs=num_idxs,
            elem_size=elem_size,
            stride_bytes_256=stride_bytes_256,
            gen_mode=0,
            single_packet=False,
            queue_num=queue_num,
            sbuf_tokens_per_rank=0,
            sbuf_free_dim_per_rank=0,
            sbuf_free_dim_pad_per_rank=0,
            sbuf_byte_offset=0,
        ))


def host_prep(inputs):
    """Canonical tile schedule shared by all cores + per-core blobs."""
    ei = np.asarray(inputs["edge_index"])
    src = np.concatenate([ei[0], np.arange(N, dtype=np.int64)]).astype(np.int64)
    dst = np.concatenate([ei[1], np.arange(N, dtype=np.int64)]).astype(np.int64)

    cores = []
    for c in range(NCORE):
        base = CS * c
        hi = min(base + CS, N)
        m = (dst >= base) & (dst < hi)
        s_c = src[m]
        r_c = (dst[m] - base).astype(np.int64)
        order = np.argsort(r_c, kind="stable")
        s_c = s_c[order]
        r_c = r_c[order]
        deg = np.bincount(r_c, minlength=CS)
        assert deg.max() <= EPT
        starts = np.zeros(CS + 1, np.int64)
        np.cumsum(deg, out=starts[1:])
        wt = {}
        for mac in range(NMACRO):
            for w in range(NW):
                lo = mac * 128 + w * WIN
                tl = []
                cur, cur_e = [], 0
                for r in range(lo, lo + WIN):
                    dd = int(deg[r])
                    if dd == 0:
                        continue
                    if cur_e + dd > EPT:
                        tl.append(cur)
                        cur, cur_e = [], 0
                    cur.append(r)
                    cur_e += dd
                if cur:
                    tl.append(cur)
                wt[(mac, w)] = tl
        cores.append(dict(s=s_c, starts=starts, wt=wt))

    tpw = np.zeros((NMACRO, NW), np.int32)
    for cc in cores:
        for (mac, w), tl in cc["wt"].items():
            tpw[mac, w] = max(tpw[mac, w], len(tl))
    sched = []  # (macro, window, first_of_window, last_of_window)
    for mac in range(NMACRO):
        for w in range(NW):
            nt = int(tpw[mac, w])
            assert nt >= 1
            for j in range(nt):
                sched.append((mac, w, j == 0, j == nt - 1))
    T = len(sched)

    per_core = []
    for c, cc in enumerate(cores):
        pat = np.zeros((128, T * WIN), np.float16)
        vs1 = np.zeros(T * 128, np.int64)   # phys1(src)
        vs2 = np.zeros(T * 128, np.int64)   # g2(src)
        vd1 = np.zeros(T * 128, np.int64)   # phys1(dst)
        vd2 = np.zeros(T * 128, np.int64)   # g2(dst)
        starts, s_c = cc["starts"], cc["s"]
        t = 0
        for mac in range(NMACRO):
            for w in range(NW):
                tl = cc["wt"][(mac, w)]
                for j in range(int(tpw[mac, w])):
                    if j < len(tl):
                        e = 0
                        for r in tl[j]:
                            dn = CS * c + r
                            for k in range(starts[r], starts[r + 1]):
                                sn = int(s_c[k])
                                pat[e, t * WIN + (r - mac * 128 - w * WIN)] = 1.0
                                vs1[t * 128 + e] = _phys1(sn)
                                vs2[t * 128 + e] = _g2(sn)
                                vd1[t * 128 + e] = _phys1(dn)
                                vd2[t * 128 + e] = _g2(dn)
                                e += 1
                    t += 1
        per_core.append(dict(pat=pat, vs1=vs1, vs2=vs2, vd1=vd1, vd2=vd2))
    return sched, per_core


def _groups_of(sched):
    mstart = [0] * (NMACRO + 1)
    for i, (mac, w, fw, lw) in enumerate(sched):
        mstart[mac + 1] = i + 1
    groups = []
    for g0 in range(0, NMACRO, GM):
        g1_ = min(g0 + GM, NMACRO)
        groups.append((mstart[g0], mstart[g1_], g0, g1_))
    return mstart, groups


def _wrap_groups(v, groups, T):
    """[T*128] values -> [32, (T+NG)*8] int16 with one zero dummy tile
    appended per group (uint16 bit-pattern encoding)."""
    NG = len(groups)
    x = v.astype(np.uint16).view(np.int16)
    out = np.zeros((16, (T + NG) * 8), np.int16)
    for gi, (t0, t1, _, _) in enumerate(groups):
        seg = x[t0 * 128:t1 * 128]
        k = np.arange(len(seg))
        out[k % 16, (t0 + gi) * 8 + k // 16] = seg
    return np.concatenate([out, out], axis=0)


def make_in_maps(inputs, sched, per_core):
    _, groups = _groups_of(sched)
    T = len(sched)
    x = np.asarray(inputs["x"], np.float32)
    xpadT = np.zeros((DIN, NODE_PAD), np.float16)
    xpadT[:, :N] = x.T.astype(np.float16)
    W1 = np.asarray(inputs["W1"], np.float32)
    perm1 = np.zeros(128, np.int64)
    for cc_ in range(C1):
        for h in range(H1):
            perm1[cc_ * H1 + h] = h * C1 + cc_
    W1p = np.ascontiguousarray(W1[:, perm1])
    a_s1 = np.asarray(inputs["att_src1"], np.float32)
    a_d1 = np.asarray(inputs["att_dst1"], np.float32)
    attv1 = np.zeros((128, 8), np.float32)
    for h in range(H1):
        for cc_ in range(C1):
            attv1[cc_ * H1 + h, h] = a_s1[h, cc_]
            attv1[cc_ * H1 + h, 4 + h] = a_d1[h, cc_]
    W2 = np.asarray(inputs["W2"], np.float32)
    W2p = np.ascontiguousarray(W2[perm1, :])
    a_s2 = np.asarray(inputs["att_src2"], np.float32)[0]
    a_d2 = np.asarray(inputs["att_dst2"], np.float32)[0]
    attv2 = np.zeros((C2, 2), np.float32)
    attv2[:, 0] = a_s2
    attv2[:, 1] = a_d2
    assert not np.any(np.asarray(inputs["bias1"])) and \
        not np.any(np.asarray(inputs["bias2"])), "nonzero bias unsupported"

    in_maps = []
    for c in range(NCORE):
        pc = per_core[c]
        in_maps.append(dict(
            xpadT=xpadT, W1p=W1p, attv1=attv1, W2p=W2p, attv2=attv2,
            pat=pc["pat"],
            is1=_wrap_groups(pc["vs1"], groups, T),
            is2=_wrap_groups(pc["vs2"], groups, T),
            id1=_wrap_groups(pc["vd1"], groups, T),
            id2=_wrap_groups(pc["vd2"], groups, T),
        ))
    return in_maps


def build_program(sched):
    T = len(sched)
    mstart, groups = _groups_of(sched)
    NG = len(groups)
    GT_MAX = max(t1 - t0 for t0, t1, _, _ in groups)
    GB = GT_MAX + 1           # SBUF blocks per group incl. dummy tile

    nc = bacc.Bacc("TRN2", target_bir_lowering=False, debug=False, num_devices=NCORE)
    tc = tile.TileContext(nc)

    xpadT_d = nc.dram_tensor("xpadT", [DIN, NODE_PAD], FP16, kind="ExternalInput")
    W1p_d = nc.dram_tensor("W1p", [128, 128], FP32, kind="ExternalInput")
    attv1_d = nc.dram_tensor("attv1", [128, 8], FP32, kind="ExternalInput")
    W2p_d = nc.dram_tensor("W2p", [128, C2], FP32, kind="ExternalInput")
    attv2_d = nc.dram_tensor("attv2", [C2, 2], FP32, kind="ExternalInput")
    pat_d = nc.dram_tensor("pat", [128, T * WIN], FP16, kind="ExternalInput")
    is1_d = nc.dram_tensor("is1", [32, (T + NG) * 8], I16, kind="ExternalInput")
    is2_d = nc.dram_tensor("is2", [32, (T + NG) * 8], I16, kind="ExternalInput")
    id1_d = nc.dram_tensor("id1", [32, (T + NG) * 8], I16, kind="ExternalInput")
    id2_d = nc.dram_tensor("id2", [32, (T + NG) * 8], I16, kind="ExternalInput")
    out2_d = nc.dram_tensor("out2", [CS, C2], FP32, kind="ExternalOutput")

    tab1_d = nc.dram_tensor("tab1i", [TB1_ROWS, R1STEP], FP16)
    tb2l_d = nc.dram_tensor("tb2li", [CS, ROW2], FP16)
    tb2g_d = nc.dram_tensor("tb2gi", [NCORE * CS, ROW2], FP16, addr_space="Shared")
    tab2_d = nc.dram_tensor("tab2i", [TB2_ROWS, R2STEP], FP16)

    with tc, ExitStack() as ctx:
        cc_sem = ctx.enter_context(nc.semaphore("ccsem"))
        const = ctx.enter_context(tc.tile_pool(name="const", bufs=1))

        w1_sb = const.tile([128, 128], FP32)
        nc.sync.dma_start(w1_sb[:], W1p_d.ap())
        attv1_sb = const.tile([128, 8], FP32)
        nc.sync.dma_start(attv1_sb[:], attv1_d.ap())
        w2_sb = const.tile([128, C2], FP32)
        nc.sync.dma_start(w2_sb[:], W2p_d.ap())
        attv2_sb = const.tile([C2, 2], FP32)
        nc.sync.dma_start(attv2_sb[:], attv2_d.ap())

        ident = const.tile([128, 128], FP32)
        make_identity(nc, ident[:])
        ident16 = const.tile([128, 128], FP16)
        make_identity(nc, ident16[:])
        nsh1 = const.tile([128, 1], FP32)
        nc.gpsimd.memset(nsh1[:], -SHIFT1)
        nsh2 = const.tile([128, 1], FP32)
        nc.gpsimd.memset(nsh2[:], -SHIFT2)

        with tc.tile_pool(name="ps0", bufs=1, space="PSUM") as ps0:
            w1t_ps = ps0.tile([128, 128], FP32, space="PSUM")
            nc.tensor.transpose(w1t_ps[:], w1_sb[:], ident[:])
            w1t_sb = const.tile([128, 128], FP32)
            nc.vector.tensor_copy(w1t_sb[:], w1t_ps[:])
            wat1_ps = ps0.tile([128, 8], FP32, space="PSUM")
            nc.tensor.matmul(wat1_ps[:], w1t_sb[:], attv1_sb[:], start=True, stop=True)
            rhs1 = const.tile([128, ROW1], FP16)
            nc.vector.tensor_copy(rhs1[:, 0:128], w1_sb[:])
            nc.vector.tensor_copy(rhs1[:, 128:ROW1], wat1_ps[:])
            w2t_ps = ps0.tile([C2, 128], FP32, space="PSUM")
            nc.tensor.transpose(w2t_ps[:], w2_sb[:], ident[:])
            w2t_sb = const.tile([C2, 128], FP32)
            nc.vector.tensor_copy(w2t_sb[:], w2t_ps[:])
            wat2_ps = ps0.tile([128, 2], FP32, space="PSUM")
            nc.tensor.matmul(wat2_ps[:], w2t_sb[:], attv2_sb[:], start=True, stop=True)
            rhs2 = const.tile([128, ROW2], FP16)
            nc.vector.tensor_copy(rhs2[:, 0:C2], w2_sb[:])
            nc.vector.tensor_copy(rhs2[:, C2:ROW2], wat2_ps[:])

        # resident data (loads overlap phase A)
        epool = ctx.enter_context(tc.tile_pool(name="edata", bufs=1))
        pat_sb = epool.tile([128, T * WIN], FP16)
        nc.sync.dma_start(pat_sb[:], pat_d.ap())
        htpool = ctx.enter_context(tc.tile_pool(name="ht", bufs=1))
        ht = htpool.tile([128, CS], FP16)

        # ---------------- phase A: tab1 = [x@W1 | a_src | a_dst] ----------
        # DRAM row of node n: phys1(n) XOR 32768  (partition p -> p^64).
        tab1v = tab1_d.ap().rearrange("(p t) c -> p t c", t=TPAD)
        with tc.tile_pool(name="xc", bufs=2) as xcp, \
             tc.tile_pool(name="pa_ps", bufs=4, space="PSUM") as paps, \
             tc.tile_pool(name="pa_st", bufs=3) as past:
            CH = 8
            for nt0 in range(0, NT, CH):
                nch = min(CH, NT - nt0)
                xc = xcp.tile([128, CH * 128], FP16, tag="xc")
                nc.sync.dma_start(xc[:, 0:nch * 128],
                                  xpadT_d.ap()[:, nt0 * 128:(nt0 + nch) * 128])
                st = past.tile([128, CH * ROW1], FP16, tag="stage")
                for k in range(nch):
                    h1ps = paps.tile([128, ROW1], FP32, space="PSUM", tag="h1")
                    nc.tensor.matmul(h1ps[:], xc[:, k * 128:(k + 1) * 128],
                                     rhs1[:], start=True, stop=True)
                    nc.scalar.activation(
                        st[:, k * ROW1:(k + 1) * ROW1], h1ps[:],
                        mybir.ActivationFunctionType.Copy)
                stv = st[:].rearrange("p (t c) -> p t c", c=ROW1)
                nc.sync.dma_start(
                    tab1v[64:128, nt0:nt0 + nch, 0:ROW1], stv[0:64, 0:nch, :])
                nc.sync.dma_start(
                    tab1v[0:64, nt0:nt0 + nch, 0:ROW1], stv[64:128, 0:nch, :])

        tc.strict_bb_all_engine_barrier()

        # ---------------- phase B: layer-1 edges ----------------
        gsrc1 = tab1_d.ap()[32768:65536, 0:F1]
        gad1 = tab1_d.ap()[32768:65536, 132:136]
        with tc.tile_pool(name="ix", bufs=2) as ixp, \
             tc.tile_pool(name="g1", bufs=2) as g1p, \
             tc.tile_pool(name="zu", bufs=2) as zup, \
             tc.tile_pool(name="msgp", bufs=2) as msgp, \
             tc.tile_pool(name="aggp", bufs=4, space="PSUM") as aggp, \
             tc.tile_pool(name="htps", bufs=2, space="PSUM") as htpsp, \
             tc.tile_pool(name="hn", bufs=2) as hnp:
            for gi, (t0, t1, m0, m1) in enumerate(groups):
                gt = t1 - t0
                o0 = (t0 + gi) * 8
                o1 = o0 + (gt + 1) * 8
                ni = (gt + 1) * 128
                ixs = ixp.tile([32, GB * 8], I16, tag="ixs")
                nc.sync.dma_start(ixs[:, 0:(gt + 1) * 8], is1_d.ap()[:, o0:o1])
                ixd = ixp.tile([32, GB * 8], I16, tag="ixd")
                nc.sync.dma_start(ixd[:, 0:(gt + 1) * 8], id1_d.ap()[:, o0:o1])
                g1 = g1p.tile([128, GB * F1], FP16, tag="g1")
                dma_gather_raw(
                    nc.gpsimd,
                    g1[:, 0:(gt + 1) * F1].rearrange("p (j c) -> p j c", c=F1),
                    gsrc1, ixs[:, 0:(gt + 1) * 8], ni, F1, R1STEP)
                adg = zup.tile([128, GB * 4], FP16, tag="ad")
                dma_gather_raw(
                    nc.gpsimd,
                    adg[:, 0:(gt + 1) * 4].rearrange("p (j c) -> p j c", c=4),
                    gad1, ixd[:, 0:(gt + 1) * 8], ni, 4, R1STEP)
                z = zup.tile([128, GB * 4], FP32, tag="z")
                nc.vector.tensor_tensor(
                    out=z[:, 0:gt * 4].rearrange("p (t h) -> p t h", h=4),
                    in0=g1[:, 0:gt * F1]
                        .rearrange("p (t c) -> p t c", c=F1)[:, :, 128:132],
                    in1=adg[:, 0:gt * 4].rearrange("p (t h) -> p t h", h=4),
                    op=mybir.AluOpType.add)
                zs = zup.tile([128, GB * 4], FP32, tag="zs")
                nc.vector.tensor_scalar_mul(zs[:, 0:gt * 4], z[:, 0:gt * 4], 0.2)
                nc.vector.tensor_tensor(out=z[:, 0:gt * 4], in0=z[:, 0:gt * 4],
                                        in1=zs[:, 0:gt * 4], op=mybir.AluOpType.max)
                u = zup.tile([128, GB * 4], FP16, tag="u")
                nc.scalar.activation(u[:, 0:gt * 4], z[:, 0:gt * 4],
                                     mybir.ActivationFunctionType.Exp, bias=nsh1[:])
                msg = msgp.tile([128, GT_MAX * F1], FP16, tag="msg")
                msgv = msg[:].rearrange("p (t c) -> p t c", c=F1)
                nc.vector.tensor_tensor(
                    out=msgv[:, 0:gt, 0:128].rearrange("p t (c h) -> p t c h", h=4),
                    in0=g1[:, 0:gt * F1]
                        .rearrange("p (t c) -> p t c", c=F1)[:, :, 0:128]
                        .rearrange("p t (c h) -> p t c h", h=4),
                    in1=u[:, 0:gt * 4].rearrange("p (t h) -> p t h", h=4)
                        .unsqueeze(2).broadcast_to([128, gt, 32, 4]),
                    op=mybir.AluOpType.mult)
                nc.vector.tensor_copy(
                    msgv[:, 0:gt, 128:132],
                    u[:, 0:gt * 4].rearrange("p (t h) -> p t h", h=4))
                for mac in range(m0, m1):
                    agg = aggp.tile([128, F1], FP32, space="PSUM", tag="agg")
                    for t in range(mstart[mac], mstart[mac + 1]):
                        _, w, fw, lw = sched[t]
                        nc.tensor.matmul(
                            agg[w * WIN:(w + 1) * WIN, :],
                            pat_sb[:, t * WIN:(t + 1) * WIN],
                            msg[:, (t - t0) * F1:(t - t0 + 1) * F1],
                            start=fw, stop=lw, tile_position=(0, w * WIN))
                    r = hnp.tile([128, 4], FP32, tag="r")
                    nc.vector.reciprocal(r[:], agg[:, 128:132])
                    hn = hnp.tile([128, 128], FP16, tag="hn")
                    nc.vector.tensor_tensor(
                        out=hn[:].rearrange("p (c h) -> p c h", h=4),
                        in0=agg[:, 0:128].rearrange("p (c h) -> p c h", h=4),
                        in1=r[:].unsqueeze(1).broadcast_to([128, 32, 4]),
                        op=mybir.AluOpType.mult)
                    a = hnp.tile([128, 128], FP16, tag="elua")
                    nc.vector.tensor_scalar_min(a[:], hn[:], 0.0)
                    e = hnp.tile([128, 128], FP16, tag="elue")
                    nc.scalar.activation(e[:], a[:], mybir.ActivationFunctionType.Exp)
                    em1 = hnp.tile([128, 128], FP16, tag="eluem")
                    nc.vector.tensor_scalar_add(em1[:], e[:], -1.0)
                    helu = hnp.tile([128, 128], FP16, tag="helu")
                    nc.vector.tensor_tensor(out=helu[:], in0=hn[:], in1=em1[:],
                                            op=mybir.AluOpType.max)
                    htps = htpsp.tile([128, 128], FP16, space="PSUM", tag="htp")
                    nc.tensor.transpose(htps[:], helu[:], ident16[:])
                    nc.scalar.activation(
                        ht[:, mac * 128:(mac + 1) * 128], htps[:],
                        mybir.ActivationFunctionType.Copy)

        tc.strict_bb_all_engine_barrier()

        # ---------------- phase C: q / tab2-local + AllGather ----------------
        with tc.tile_pool(name="pc_ps", bufs=6, space="PSUM") as pcps, \
             tc.tile_pool(name="pc_st", bufs=3) as pcst:
            CH2 = 8
            for sc0 in range(0, NMACRO, CH2):
                nch = min(CH2, NMACRO - sc0)
                st2 = pcst.tile([128, CH2 * ROW2], FP16, tag="st2")
                for k in range(nch):
                    scc = sc0 + k
                    qps = pcps.tile([128, ROW2], FP32, space="PSUM", tag="q")
                    nc.tensor.matmul(qps[:], ht[:, scc * 128:(scc + 1) * 128],
                                     rhs2[:], start=True, stop=True)
                    nc.scalar.activation(
                        st2[:, k * ROW2:(k + 1) * ROW2], qps[:],
                        mybir.ActivationFunctionType.Copy)
                st2v = st2[:].rearrange("p (t c) -> p t c", c=ROW2)
                dst2 = tb2l_d.ap().rearrange("(p t) c -> p t c", t=NMACRO)
                nc.sync.dma_start(
                    dst2[:, sc0:sc0 + nch, :], st2v[:, 0:nch, :])

        tc.strict_bb_all_engine_barrier()
        with tc.tile_critical():
            nc.gpsimd.collective_compute(
                "AllGather", mybir.AluOpType.bypass,
                replica_groups=[list(range(NCORE))],
                ins=[tb2l_d.ap().opt()],
                outs=[tb2g_d.ap().opt()],
            ).then_inc(cc_sem)
            nc.gpsimd.wait_ge(cc_sem, 1)
        tc.strict_bb_all_engine_barrier()
        # reshape dense AllGather output into 256B-stride gather layout
        # (row g -> g XOR 32768)
        nc.sync.dma_start(tab2_d.ap()[32768:65536, 0:ROW2],
                          tb2g_d.ap()[0:32768, :])
        nc.sync.dma_start(tab2_d.ap()[0:NCORE * CS - 32768, 0:ROW2],
                          tb2g_d.ap()[32768:NCORE * CS, :])
        tc.strict_bb_all_engine_barrier()

        # ---------------- phase D: layer-2 edges ----------------
        gsrc2 = tab2_d.ap()[32768:65536, 0:ROW2S]
        gad2 = tab2_d.ap()[32768:65536, 33:34]
        opool = ctx.enter_context(tc.tile_pool(name="oacc", bufs=1))
        oacc = opool.tile([128, NMACRO * C2], FP32)
        with tc.tile_pool(name="ix2", bufs=2) as ixp2, \
             tc.tile_pool(name="g2", bufs=2) as g2p, \
             tc.tile_pool(name="zu2", bufs=2) as zup2, \
             tc.tile_pool(name="msg2", bufs=2) as msgp2, \
             tc.tile_pool(name="agg2", bufs=6, space="PSUM") as aggp2, \
             tc.tile_pool(name="o2", bufs=2) as o2p:
            for gi, (t0, t1, m0, m1) in enumerate(groups):
                gt = t1 - t0
                o0 = (t0 + gi) * 8
                o1 = o0 + (gt + 1) * 8
                ni = (gt + 1) * 128
                ixs = ixp2.tile([32, GB * 8], I16, tag="ixs2")
                nc.sync.dma_start(ixs[:, 0:(gt + 1) * 8], is2_d.ap()[:, o0:o1])
                ixd = ixp2.tile([32, GB * 8], I16, tag="ixd2")
                nc.sync.dma_start(ixd[:, 0:(gt + 1) * 8], id2_d.ap()[:, o0:o1])
                g2 = g2p.tile([128, GB * ROW2S], FP16, tag="g2")
                dma_gather_raw(
                    nc.gpsimd,
                    g2[:, 0:(gt + 1) * ROW2S].rearrange("p (j c) -> p j c", c=ROW2S),
                    gsrc2, ixs[:, 0:(gt + 1) * 8], ni, ROW2S, R2STEP)
                ad2 = zup2.tile([128, GB], FP16, tag="ad2")
                dma_gather_raw(
                    nc.gpsimd,
                    ad2[:, 0:gt + 1].rearrange("p (j c) -> p j c", c=1),
                    gad2, ixd[:, 0:(gt + 1) * 8], ni, 1, R2STEP)
                g2v = g2[:].rearrange("p (t c) -> p t c", c=ROW2S)
                z2 = zup2.tile([128, GB], FP32, tag="z2")
                nc.vector.tensor_tensor(
                    out=z2[:, 0:gt].unsqueeze(2),
                    in0=g2v[:, 0:gt, 32:33],
                    in1=ad2[:, 0:gt].unsqueeze(2),
                    op=mybir.AluOpType.add)
                zs2 = zup2.tile([128, GB], FP32, tag="zs2")
                nc.vector.tensor_scalar_mul(zs2[:, 0:gt], z2[:, 0:gt], 0.2)
                nc.vector.tensor_tensor(out=z2[:, 0:gt], in0=z2[:, 0:gt],
                                        in1=zs2[:, 0:gt], op=mybir.AluOpType.max)
                u2 = zup2.tile([128, GT_MAX * 2], FP16, tag="u2")
                u2v = u2[:].rearrange("p (t j) -> p t j", j=2)
                nc.scalar.activation(u2v[:, 0:gt, 0:1], z2[:, 0:gt].unsqueeze(2),
                                     mybir.ActivationFunctionType.Exp, bias=nsh2[:])
                nc.vector.tensor_copy(u2v[:, 0:gt, 1:2], u2v[:, 0:gt, 0:1])
                msg2 = msgp2.tile([128, GT_MAX * 34], FP16, tag="m2")
                m2v = msg2[:].rearrange("p (t c) -> p t c", c=34)
                nc.vector.tensor_tensor(
                    out=m2v[:, 0:gt, 0:32].rearrange("p t (c j) -> p t c j", j=2),
                    in0=g2v[:, 0:gt, 0:32].rearrange("p t (c j) -> p t c j", j=2),
                    in1=u2v[:, 0:gt, :].unsqueeze(2).broadcast_to([128, gt, 16, 2]),
                    op=mybir.AluOpType.mult)
                nc.vector.tensor_copy(m2v[:, 0:gt, 32:34], u2v[:, 0:gt, :])
                for mac in range(m0, m1):
                    agg2 = aggp2.tile([128, F2], FP32, space="PSUM", tag="ag2")
                    for t in range(mstart[mac], mstart[mac + 1]):
                        _, w, fw, lw = sched[t]
                        nc.tensor.matmul(
                            agg2[w * WIN:(w + 1) * WIN, :],
                            pat_sb[:, t * WIN:(t + 1) * WIN],
                            msg2[:, (t - t0) * 34:(t - t0) * 34 + F2],
                            start=fw, stop=lw, tile_position=(0, w * WIN))
                    r2 = o2p.tile([128, 1], FP32, tag="r2")
                    nc.vector.reciprocal(r2[:], agg2[:, C2:C2 + 1])
                    nc.vector.tensor_tensor(
                        out=oacc[:, mac * C2:(mac + 1) * C2], in0=agg2[:, 0:C2],
                        in1=r2[:].broadcast_to([128, C2]),
                        op=mybir.AluOpType.mult)
            nc.sync.dma_start(
                out2_d.ap().rearrange("(m p) c -> p m c", p=128),
                oacc[:].rearrange("p (m c) -> p m c", c=C2))

    nc.compile()
    return nc


_CACHE = {}


def run(inputs, trace=False):
    sched, per_core = host_prep(inputs)
    in_maps = make_in_maps(inputs, sched, per_core)
    key = len(sched)
    if key not in _CACHE:
        _CACHE[key] = build_program(sched)
    nc = _CACHE[key]
    res = run_bass_kernel_spmd(nc, in_maps, core_ids=list(range(NCORE)), trace=trace)
    outs = [r["out2"] for r in res.results]
    out = np.zeros((N, C2), np.float32)
    for c in range(NCORE):
        lo = c * CS
        hi = min(lo + CS, N)
        out[lo:hi] = outs[c][: hi - lo]
    return out, res


def kernel(**inputs):
    """Full-input GAT kernel: shards across 8 NeuronCores internally."""
    import numpy as _np
    out, _res = run(inputs)
    return out.astype(_np.float32)


# revision 14
# speedup vs baseline: 2.1114x; 1.6295x over previous
# GAT 2-layer kernel for trn2 x8 — host prep + bass program + runner.
# Sharding: dst-node partition (graph parallel). Each core owns a contiguous
# 6272-node block and processes all edges into it; layer-1 node features are
# computed replicated; layer-2 node tables are exchanged via AllGather.
#
# Edge gathers use InstDMAGatherAnt split into <=1024-index sub-ops round-
# robined over 4 SWDGE queues. Tables larger than the int16 index range use
# a sign-wrap trick: indices are uint16 bit patterns, the gather base sits
# at +32768 rows, rows are stored at physical_row XOR 32768. Each group's
# index block ends with one all-zero dummy tile, and sub-op boundary slots
# are swapped to keep trailing indices non-negative (the ucode trims
# trailing negatives).
#
# Dst-attention values are NOT gathered per edge: they are expanded on the
# TensorEngine via per-tile patT @ ad_window matmuls (patT host-supplied,
# streamed per group; window ads staged in SBUF by a small bootstrap gather
# plus 4 selection matmuls).
from contextlib import ExitStack

import numpy as np

import concourse.bass as bass
import concourse.bacc as bacc
import concourse.tile as tile
from concourse import mybir
from concourse.bass_utils import run_bass_kernel_spmd
from concourse.masks import make_identity

# ---- problem constants ----
N = 50000
DIN = 128
H1, C1 = 4, 32
C2 = 32
NCORE = 8

NODE_PAD = 50048          # 391 * 128
NT = NODE_PAD // 128      # 391 node tiles
TPAD = 512                # padded tiles-per-partition in tab1 row index
CS = 6272                 # slots per core (49*128); node n -> core n//CS
NMACRO = CS // 128        # 49
WIN = 32                  # dst-window width (psum base must be 32-aligned)
NW = 128 // WIN
EPT = 128                 # max edges per tile
GM = 3                    # macros per gather group
SUB = 1024                # indices per dma_gather sub-op

ROW1 = 136                # tab1 row: [h (c,h)-interleaved 128 | a_src 4 | a_dst 4]
R1STEP = 256              # tab1 row stride in fp16 elems (512 B)
F1 = 132                  # gathered src row / msg width: [h 128 | a_src 4]
ROW2 = 34                 # tab2 row: [q 32 | a_src | a_dst]
ROW2S = 33                # gathered layer-2 src row: [q 32 | a_src]
R2STEP = 128              # tab2 row stride in fp16 elems (256 B)
F2 = 33                   # layer-2 msg width: 32 feats + u
SHIFT1 = 6.0
SHIFT2 = 2.0

TB1_ROWS = 65536          # tab1 rows (stored at phys1 XOR 32768)
TB2_ROWS = 65536          # tab2 rows (stored at g2 XOR 32768); g2 in [0, 50176)
NLOC = NMACRO * 128       # 6272 local slots
ILOC_PAD = (NLOC // 16) + 8   # bootstrap idx cols incl. one dummy tile

FP16 = mybir.dt.float16
FP32 = mybir.dt.float32
I32 = mybir.dt.int32
I16 = mybir.dt.int16


def _phys1(n):
    return (n % 128) * TPAD + (n // 128)


def _g2(n):
    c, s = n // CS, n % CS
    return c * CS + (s % 128) * NMACRO + (s // 128)


def dma_gather_raw(g, out_ap, in_ap, idxs_ap, num_idxs, elem_size, elem_step,
                   queue_num=0):
    """bass.dma_gather without the elem_size%256 assert (non-transpose)."""
    from concourse import ap_utils
    from concourse.bass import exact_div
    assert idxs_ap.dtype == mybir.dt.int16
    assert in_ap.dtype == out_ap.dtype
    assert ap_utils.ap_is_contiguous(out_ap.ap[1:])
    assert ap_utils.ap_is_contiguous(idxs_ap.ap[1:])
    assert in_ap.ap[-1][1] == out_ap.ap[-1][1] == elem_size
    assert in_ap.ap[0][0] == elem_step
    stride_bytes = elem_step * mybir.dt.size(in_ap.dtype)
    stride_bytes_256 = exact_div(stride_bytes, 256)
    _in_ap = g.lower_ap_dma(in_ap, for_custom_bir_dma=True)
    _idxs_ap = g.lower_ap(idxs_ap)
    _out_ap = g.lower_ap(out_ap)
    return g.add_instruction(
        mybir.InstDMAGatherAnt(
            name=g.bass.get_next_instruction_name(),
            ins=[*_in_ap, _idxs_ap, g.lower_val_access(g.to_reg(num_idxs))],
            outs=[_out_ap],
            transpose=False,
            num_idxs=num_idxs,
            elem_size=elem_size,
            stride_bytes_256=stride_bytes_256,
            gen_mode=0,
            single_packet=False,
            queue_num=queue_num,
            sbuf_tokens_per_rank=0,
            sbuf_free_dim_per_rank=0,
            sbuf_free_dim_pad_per_rank=0,
            sbuf_byte_offset=0,
        ))


class _QRot:
    def __init__(self, nq=4):
        self.i = 0
        self.nq = nq

    def __call__(self):
        q = self.i % self.nq
        self.i += 1
        return q


def host_prep(inputs):
    """Canonical tile schedule shared by all cores + per-core blobs."""
    ei = np.asarray(inputs["edge_index"])
    src = np.concatenate([ei[0], np.arange(N, dtype=np.int64)]).astype(np.int64)
    dst = np.concatenate([ei[1], np.arange(N, dtype=np.int64)]).astype(np.int64)

    cores = []
    for c in range(NCORE):
        base = CS * c
        hi = min(base + CS, N)
        m = (dst >= base) & (dst < hi)
        s_c = src[m]
        r_c = (dst[m] - base).astype(np.int64)
        order = np.argsort(r_c, kind="stable")
        s_c = s_c[order]
        r_c = r_c[order]
        deg = np.bincount(r_c, minlength=CS)
        assert deg.max() <= EPT
        starts = np.zeros(CS + 1, np.int64)
        np.cumsum(deg, out=starts[1:])
        wt = {}
        for mac in range(NMACRO):
            for w in range(NW):
                lo = mac * 128 + w * WIN
                tl = []
                cur, cur_e = [], 0
                for r in range(lo, lo + WIN):
                    dd = int(deg[r])
                    if dd == 0:
                        continue
                    if cur_e + dd > EPT:
                        tl.append(cur)
                        cur, cur_e = [], 0
                    cur.append(r)
                    cur_e += dd
                if cur:
                    tl.append(cur)
                wt[(mac, w)] = tl
        cores.append(dict(s=s_c, starts=starts, wt=wt))

    tpw = np.zeros((NMACRO, NW), np.int32)
    for cc in cores:
        for (mac, w), tl in cc["wt"].items():
            tpw[mac, w] = max(tpw[mac, w], len(tl))
    sched = []  # (macro, window, first_of_window, last_of_window)
    for mac in range(NMACRO):
        for w in range(NW):
            nt = int(tpw[mac, w])
            assert nt >= 1
            for j in range(nt):
                sched.append((mac, w, j == 0, j == nt - 1))
    T = len(sched)
    _, groups = _groups_of(sched)

    per_core = []
    for c, cc in enumerate(cores):
        pat = np.zeros((128, T * WIN), np.float16)
        vs1 = np.zeros(T * 128, np.int64)   # phys1(src)
        vs2 = np.zeros(T * 128, np.int64)   # g2(src)
        starts, s_c = cc["starts"], cc["s"]
        t = 0
        for mac in range(NMACRO):
            for w in range(NW):
                tl = cc["wt"][(mac, w)]
                for j in range(int(tpw[mac, w])):
                    if j < len(tl):
                        e = 0
                        for r in tl[j]:
                            for k in range(starts[r], starts[r + 1]):
                                sn = int(s_c[k])
                                pat[e, t * WIN + (r - mac * 128 - w * WIN)] = 1.0
                                vs1[t * 128 + e] = _phys1(sn)
                                vs2[t * 128 + e] = _g2(sn)
                                e += 1
                    t += 1

        # keep trailing index of every SUB-sized sub-op non-negative as
        # int16 (uint16 < 32768) for BOTH src tables: swap within the tile.
        for (t0, t1, _, _) in groups:
            span = (t1 - t0 + 1) * 128
            for kb in range(SUB - 1, span, SUB):
                trel = kb // 128
                if trel >= t1 - t0:
                    continue  # dummy pad tile (zeros)
                tt = t0 + trel
                last = tt * 128 + 127
                if vs1[last] < 32768 and vs2[last] < 32768:
                    continue
                lo = tt * 128
                cand = np.nonzero((vs1[lo:last] < 32768)
                                  & (vs2[lo:last] < 32768))[0]
                assert len(cand), "no swappable slot in sub-op-final tile"
                kk = lo + cand[-1]
                for a in (vs1, vs2):
                    a[kk], a[last] = a[last], a[kk]
                e1, e2 = kk - lo, 127
                tcol = slice(tt * WIN, (tt + 1) * WIN)
                tmp = pat[e1, tcol].copy()
                pat[e1, tcol] = pat[e2, tcol]
                pat[e2, tcol] = tmp

        # patT[d, t*128+e] = pat[e, t*32+d]
        patT = np.zeros((32, T * 128), np.float16)
        for t in range(T):
            patT[:, t * 128:(t + 1) * 128] = pat[:, t * WIN:(t + 1) * WIN].T

        # bootstrap idx: phys1 of own local slots with partition bit 6
        # flipped (so every 1024-boundary index is non-negative as int16),
        # one dummy tile at end
        vloc = np.array(
            [_phys1(c * CS + (s // 128) * 128 + ((s % 128) ^ 64))
             for s in range(NLOC)], np.int64)
        iloc = np.zeros((16, ILOC_PAD), np.int16)
        kk = np.arange(NLOC)
        iloc[kk % 16, kk // 16] = vloc.astype(np.uint16).view(np.int16)
        assert all(vloc[k] < 32768 for k in range(SUB - 1, NLOC, SUB))
        iloc = np.concatenate([iloc, iloc], axis=0)  # [32, ILOC_PAD]

        per_core.append(dict(pat=pat, patT=patT, vs1=vs1, vs2=vs2, iloc=iloc))
    return sched, per_core


def _groups_of(sched):
    mstart = [0] * (NMACRO + 1)
    for i, (mac, w, fw, lw) in enumerate(sched):
        mstart[mac + 1] = i + 1
    groups = []
    for g0 in range(0, NMACRO, GM):
        g1_ = min(g0 + GM, NMACRO)
        groups.append((mstart[g0], mstart[g1_], g0, g1_))
    return mstart, groups


def _wrap_groups(v, groups, T):
    """[T*128] values -> [32, (T+NG)*8] int16 with one zero dummy tile
    appended per group (uint16 bit-pattern encoding)."""
    NG = len(groups)
    x = v.astype(np.uint16).view(np.int16)
    out = np.zeros((16, (T + NG) * 8), np.int16)
    for gi, (t0, t1, _, _) in enumerate(groups):
        seg = x[t0 * 128:t1 * 128]
        k = np.arange(len(seg))
        out[k % 16, (t0 + gi) * 8 + k // 16] = seg
    return np.concatenate([out, out], axis=0)


def make_in_maps(inputs, sched, per_core):
    _, groups = _groups_of(sched)
    T = len(sched)
    x = np.asarray(inputs["x"], np.float32)
    xpadT = np.zeros((DIN, NODE_PAD), np.float16)
    xpadT[:, :N] = x.T.astype(np.float16)
    W1 = np.asarray(inputs["W1"], np.float32)
    perm1 = np.zeros(128, np.int64)
    for cc_ in range(C1):
        for h in range(H1):
            perm1[cc_ * H1 + h] = h * C1 + cc_
    W1p = np.ascontiguousarray(W1[:, perm1])
    a_s1 = np.asarray(inputs["att_src1"], np.float32)
    a_d1 = np.asarray(inputs["att_dst1"], np.float32)
    attv1 = np.zeros((128, 8), np.float32)
    for h in range(H1):
        for cc_ in range(C1):
            attv1[cc_ * H1 + h, h] = a_s1[h, cc_]
            attv1[cc_ * H1 + h, 4 + h] = a_d1[h, cc_]
    W2 = np.asarray(inputs["W2"], np.float32)
    W2p = np.ascontiguousarray(W2[perm1, :])
    a_s2 = np.asarray(inputs["att_src2"], np.float32)[0]
    a_d2 = np.asarray(inputs["att_dst2"], np.float32)[0]
    attv2 = np.zeros((C2, 2), np.float32)
    attv2[:, 0] = a_s2
    attv2[:, 1] = a_d2
    assert not np.any(np.asarray(inputs["bias1"])) and \
        not np.any(np.asarray(inputs["bias2"])), "nonzero bias unsupported"

    # selW cols 0:512 (layer 1, un-flips the bootstrap's partition bit):
    #   selW[i, w*128+j] = 1 iff i == (w*32 + j%32) ^ 64
    # selW cols 512:1024 (layer 2, plain):
    #   selW[i, 512 + w*128+j] = 1 iff i == w*32 + j%32
    selW = np.zeros((128, 8 * 128), np.float16)
    for w in range(4):
        for j in range(128):
            selW[(w * 32 + (j % 32)) ^ 64, w * 128 + j] = 1.0
            selW[w * 32 + (j % 32), 512 + w * 128 + j] = 1.0

    in_maps = []
    for c in range(NCORE):
        pc = per_core[c]
        in_maps.append(dict(
            xpadT=xpadT, W1p=W1p, attv1=attv1, W2p=W2p, attv2=attv2,
            selW=selW, pat=pc["pat"], patT=pc["patT"], iloc=pc["iloc"],
            is1=_wrap_groups(pc["vs1"], groups, T),
            is2=_wrap_groups(pc["vs2"], groups, T),
        ))
    return in_maps


def build_program(sched):
    T = len(sched)
    mstart, groups = _groups_of(sched)
    NG = len(groups)
    GT_MAX = max(t1 - t0 for t0, t1, _, _ in groups)
    GB = GT_MAX + 1           # SBUF blocks per group incl. dummy tile

    nc = bacc.Bacc("TRN2", target_bir_lowering=False, debug=False,
                   num_devices=NCORE, dynamic_dma_scratch_size=16384,
                   num_swdge_queues=1)
    tc = tile.TileContext(nc)

    xpadT_d = nc.dram_tensor("xpadT", [DIN, NODE_PAD], FP16, kind="ExternalInput")
    W1p_d = nc.dram_tensor("W1p", [128, 128], FP32, kind="ExternalInput")
    attv1_d = nc.dram_tensor("attv1", [128, 8], FP32, kind="ExternalInput")
    W2p_d = nc.dram_tensor("W2p", [128, C2], FP32, kind="ExternalInput")
    attv2_d = nc.dram_tensor("attv2", [C2, 2], FP32, kind="ExternalInput")
    selW_d = nc.dram_tensor("selW", [128, 8 * 128], FP16, kind="ExternalInput")
    pat_d = nc.dram_tensor("pat", [128, T * WIN], FP16, kind="ExternalInput")
    patT_d = nc.dram_tensor("patT", [32, T * 128], FP16, kind="ExternalInput")
    iloc_d = nc.dram_tensor("iloc", [32, ILOC_PAD], I16, kind="ExternalInput")
    is1_d = nc.dram_tensor("is1", [32, (T + NG) * 8], I16, kind="ExternalInput")
    is2_d = nc.dram_tensor("is2", [32, (T + NG) * 8], I16, kind="ExternalInput")
    out2_d = nc.dram_tensor("out2", [CS, C2], FP32, kind="ExternalOutput")

    tab1_d = nc.dram_tensor("tab1i", [TB1_ROWS, R1STEP], FP16)
    tb2l_d = nc.dram_tensor("tb2li", [CS, ROW2], FP16)
    tb2g_d = nc.dram_tensor("tb2gi", [NCORE * CS, ROW2], FP16, addr_space="Shared")
    tab2_d = nc.dram_tensor("tab2i", [TB2_ROWS, R2STEP], FP16)

    qrot = _QRot(1)

    with tc, ExitStack() as ctx:
        cc_sem = ctx.enter_context(nc.semaphore("ccsem"))
        const = ctx.enter_context(tc.tile_pool(name="const", bufs=1))

        w1_sb = const.tile([128, 128], FP32)
        nc.sync.dma_start(w1_sb[:], W1p_d.ap())
        attv1_sb = const.tile([128, 8], FP32)
        nc.sync.dma_start(attv1_sb[:], attv1_d.ap())
        w2_sb = const.tile([128, C2], FP32)
        nc.sync.dma_start(w2_sb[:], W2p_d.ap())
        attv2_sb = const.tile([C2, 2], FP32)
        nc.sync.dma_start(attv2_sb[:], attv2_d.ap())
        selw_sb = const.tile([128, 8 * 128], FP16)
        nc.sync.dma_start(selw_sb[:], selW_d.ap())

        ident = const.tile([128, 128], FP32)
        make_identity(nc, ident[:])
        ident16 = const.tile([128, 128], FP16)
        make_identity(nc, ident16[:])
        nsh1 = const.tile([128, 1], FP32)
        nc.gpsimd.memset(nsh1[:], -SHIFT1)
        nsh2 = const.tile([128, 1], FP32)
        nc.gpsimd.memset(nsh2[:], -SHIFT2)

        with tc.tile_pool(name="ps0", bufs=1, space="PSUM") as ps0:
            w1t_ps = ps0.tile([128, 128], FP32, space="PSUM")
            nc.tensor.transpose(w1t_ps[:], w1_sb[:], ident[:])
            w1t_sb = const.tile([128, 128], FP32)
            nc.vector.tensor_copy(w1t_sb[:], w1t_ps[:])
            wat1_ps = ps0.tile([128, 8], FP32, space="PSUM")
            nc.tensor.matmul(wat1_ps[:], w1t_sb[:], attv1_sb[:], start=True, stop=True)
            rhs1 = const.tile([128, ROW1], FP16)
            nc.vector.tensor_copy(rhs1[:, 0:128], w1_sb[:])
            nc.vector.tensor_copy(rhs1[:, 128:ROW1], wat1_ps[:])
            w2t_ps = ps0.tile([C2, 128], FP32, space="PSUM")
            nc.tensor.transpose(w2t_ps[:], w2_sb[:], ident[:])
            w2t_sb = const.tile([C2, 128], FP32)
            nc.vector.tensor_copy(w2t_sb[:], w2t_ps[:])
            wat2_ps = ps0.tile([128, 2], FP32, space="PSUM")
            nc.tensor.matmul(wat2_ps[:], w2t_sb[:], attv2_sb[:], start=True, stop=True)
            rhs2 = const.tile([128, ROW2], FP16)
            nc.vector.tensor_copy(rhs2[:, 0:C2], w2_sb[:])
            nc.vector.tensor_copy(rhs2[:, C2:ROW2], wat2_ps[:])

        htpool = ctx.enter_context(tc.tile_pool(name="ht", bufs=1))
        ht = htpool.tile([128, CS], FP16)
        adpool = ctx.enter_context(tc.tile_pool(name="adp", bufs=1))
        adslw = adpool.tile([128, 4 * NMACRO * 4], FP16)   # layer-1 window ads
        adsl2 = adpool.tile([128, NMACRO], FP16)           # layer-2 per-slot ad
        adsl2w = adpool.tile([128, 4 * NMACRO], FP16)      # layer-2 window ads

        # ---------------- phase A: tab1 = [x@W1 | a_src | a_dst] ----------
        # DRAM row of node n: phys1(n) XOR 32768  (partition p -> p^64).
        tab1v = tab1_d.ap().rearrange("(p t) c -> p t c", t=TPAD)
        with tc.tile_pool(name="xc", bufs=2) as xcp, \
             tc.tile_pool(name="pa_ps", bufs=4, space="PSUM") as paps, \
             tc.tile_pool(name="pa_st", bufs=3) as past:
            CH = 8
            for nt0 in range(0, NT, CH):
                nch = min(CH, NT - nt0)
                xc = xcp.tile([128, CH * 128], FP16, tag="xc")
                nc.sync.dma_start(xc[:, 0:nch * 128],
                                  xpadT_d.ap()[:, nt0 * 128:(nt0 + nch) * 128])
                st = past.tile([128, CH * ROW1], FP16, tag="stage")
                for k in range(nch):
                    h1ps = paps.tile([128, ROW1], FP32, space="PSUM", tag="h1")
                    nc.tensor.matmul(h1ps[:], xc[:, k * 128:(k + 1) * 128],
                                     rhs1[:], start=True, stop=True)
                    nc.scalar.activation(
                        st[:, k * ROW1:(k + 1) * ROW1], h1ps[:],
                        mybir.ActivationFunctionType.Copy)
                stv = st[:].rearrange("p (t c) -> p t c", c=ROW1)
                nc.sync.dma_start(
                    tab1v[64:128, nt0:nt0 + nch, 0:ROW1], stv[0:64, 0:nch, :])
                nc.sync.dma_start(
                    tab1v[0:64, nt0:nt0 + nch, 0:ROW1], stv[64:128, 0:nch, :])

        tc.strict_bb_all_engine_barrier()

        # ---------------- bootstrap: local window a_dst into SBUF ----------
        gad1 = tab1_d.ap()[32768:65536, 132:136]
        with tc.tile_pool(name="boot", bufs=1) as bpool, \
             tc.tile_pool(name="boot_ps", bufs=2, space="PSUM") as bps:
            iloc_sb = bpool.tile([32, ILOC_PAD], I16)
            nc.sync.dma_start(iloc_sb[:], iloc_d.ap())
            adsl_g = bpool.tile([128, (NMACRO + 1) * 4], FP16)
            for s0 in range(0, NLOC + 1, SUB):
                n = min(SUB, NLOC + 1 - s0)
                dma_gather_raw(
                    nc.gpsimd,
                    adsl_g[:, (s0 // 128) * 4:((s0 + n + 127) // 128) * 4]
                    .rearrange("p (j c) -> p j c", c=4),
                    gad1, iloc_sb[:, s0 // 16:(s0 + n + 15) // 16], n, 4,
                    R1STEP, queue_num=qrot())
            for w in range(4):
                adw_ps = bps.tile([128, NMACRO * 4], FP32, space="PSUM",
                                  tag="adw")
                nc.tensor.matmul(adw_ps[:],
                                 selw_sb[:, w * 128:(w + 1) * 128],
                                 adsl_g[:, 0:NMACRO * 4], start=True, stop=True)
                nc.scalar.activation(
                    adslw[:, w * NMACRO * 4:(w + 1) * NMACRO * 4], adw_ps[:],
                    mybir.ActivationFunctionType.Copy)

        # ---------------- phase B: layer-1 edges ----------------
        gsrc1 = tab1_d.ap()[32768:65536, 0:F1]
        with tc.tile_pool(name="ix", bufs=2) as ixp, \
             tc.tile_pool(name="pt", bufs=2) as ptp, \
             tc.tile_pool(name="g1", bufs=2) as g1p, \
             tc.tile_pool(name="zu", bufs=2) as zup, \
             tc.tile_pool(name="msgp", bufs=2) as msgp, \
             tc.tile_pool(name="adx_ps", bufs=2, space="PSUM") as adxp, \
             tc.tile_pool(name="aggp", bufs=3, space="PSUM") as aggp, \
             tc.tile_pool(name="htps", bufs=1, space="PSUM") as htpsp, \
             tc.tile_pool(name="hn", bufs=2) as hnp:
            for gi, (t0, t1, m0, m1) in enumerate(groups):
                gt = t1 - t0
                o0 = (t0 + gi) * 8
                ni = (gt + 1) * 128
                ixs = ixp.tile([32, GB * 8], I16, tag="ixs")
                nc.sync.dma_start(ixs[:, 0:(gt + 1) * 8],
                                  is1_d.ap()[:, o0:o0 + (gt + 1) * 8])
                patg = ptp.tile([128, GT_MAX * WIN], FP16, tag="patg")
                nc.sync.dma_start(patg[:, 0:gt * WIN],
                                  pat_d.ap()[:, t0 * WIN:t1 * WIN])
                patTg = ptp.tile([32, GT_MAX * 128], FP16, tag="patTg")
                nc.sync.dma_start(patTg[:, 0:gt * 128],
                                  patT_d.ap()[:, t0 * 128:t1 * 128])
                g1 = g1p.tile([128, GB * F1], FP16, tag="g1")
                for s0 in range(0, ni, SUB):
                    n = min(SUB, ni - s0)
                    dma_gather_raw(
                        nc.gpsimd,
                        g1[:, (s0 // 128) * F1:((s0 + n) // 128) * F1]
                        .rearrange("p (j c) -> p j c", c=F1),
                        gsrc1, ixs[:, s0 // 16:(s0 + n) // 16], n, F1, R1STEP,
                        queue_num=qrot())
                # expand dst-att on PE: adx[:, t*4:(t+1)*4] = patT_t^T @ adwin
                adx = adxp.tile([128, GT_MAX * 4], FP32, space="PSUM", tag="adx")
                for t in range(t0, t1):
                    mac, w, _, _ = sched[t]
                    nc.tensor.matmul(
                        adx[:, (t - t0) * 4:(t - t0 + 1) * 4],
                        patTg[:, (t - t0) * 128:(t - t0 + 1) * 128],
                        adslw[0:32, (w * NMACRO + mac) * 4:
                              (w * NMACRO + mac + 1) * 4],
                        start=True, stop=True)
                z = zup.tile([128, GT_MAX * 4], FP32, tag="z")
                nc.vector.tensor_tensor(
                    out=z[:, 0:gt * 4].rearrange("p (t h) -> p t h", h=4),
                    in0=g1[:, 0:gt * F1]
                        .rearrange("p (t c) -> p t c", c=F1)[:, :, 128:132],
                    in1=adx[:, 0:gt * 4].rearrange("p (t h) -> p t h", h=4),
                    op=mybir.AluOpType.add)
                zs = zup.tile([128, GT_MAX * 4], FP32, tag="zs")
                nc.vector.tensor_scalar_mul(zs[:, 0:gt * 4], z[:, 0:gt * 4], 0.2)
                nc.vector.tensor_tensor(out=z[:, 0:gt * 4], in0=z[:, 0:gt * 4],
                                        in1=zs[:, 0:gt * 4], op=mybir.AluOpType.max)
                u = zup.tile([128, GT_MAX * 4], FP16, tag="u")
                nc.scalar.activation(u[:, 0:gt * 4], z[:, 0:gt * 4],
                                     mybir.ActivationFunctionType.Exp, bias=nsh1[:])
                msg = msgp.tile([128, GT_MAX * F1], FP16, tag="msg")
                msgv = msg[:].rearrange("p (t c) -> p t c", c=F1)
                nc.vector.tensor_tensor(
                    out=msgv[:, 0:gt, 0:128].rearrange("p t (c h) -> p t c h", h=4),
                    in0=g1[:, 0:gt * F1]
                        .rearrange("p (t c) -> p t c", c=F1)[:, :, 0:128]
                        .rearrange("p t (c h) -> p t c h", h=4),
                    in1=u[:, 0:gt * 4].rearrange("p (t h) -> p t h", h=4)
                        .unsqueeze(2).broadcast_to([128, gt, 32, 4]),
                    op=mybir.AluOpType.mult)
                nc.vector.tensor_copy(
                    msgv[:, 0:gt, 128:132],
                    u[:, 0:gt * 4].rearrange("p (t h) -> p t h", h=4))
                for mac in range(m0, m1):
                    agg = aggp.tile([128, F1], FP32, space="PSUM", tag="agg")
                    for t in range(mstart[mac], mstart[mac + 1]):
                        _, w, fw, lw = sched[t]
                        nc.tensor.matmul(
                            agg[w * WIN:(w + 1) * WIN, :],
                            patg[:, (t - t0) * WIN:(t - t0 + 1) * WIN],
                            msg[:, (t - t0) * F1:(t - t0 + 1) * F1],
                            start=fw, stop=lw, tile_position=(0, w * WIN))
                    r = hnp.tile([128, 4], FP32, tag="r")
                    nc.vector.reciprocal(r[:], agg[:, 128:132])
                    hn = hnp.tile([128, 128], FP16, tag="hn")
                    nc.vector.tensor_tensor(
                        out=hn[:].rearrange("p (c h) -> p c h", h=4),
                        in0=agg[:, 0:128].rearrange("p (c h) -> p c h", h=4),
                        in1=r[:].unsqueeze(1).broadcast_to([128, 32, 4]),
                        op=mybir.AluOpType.mult)
                    a = hnp.tile([128, 128], FP16, tag="elua")
                    nc.vector.tensor_scalar_min(a[:], hn[:], 0.0)
                    e = hnp.tile([128, 128], FP16, tag="elue")
                    nc.scalar.activation(e[:], a[:], mybir.ActivationFunctionType.Exp)
                    em1 = hnp.tile([128, 128], FP16, tag="eluem")
                    nc.vector.tensor_scalar_add(em1[:], e[:], -1.0)
                    helu = hnp.tile([128, 128], FP16, tag="helu")
                    nc.vector.tensor_tensor(out=helu[:], in0=hn[:], in1=em1[:],
                                            op=mybir.AluOpType.max)
                    htps = htpsp.tile([128, 128], FP16, space="PSUM", tag="htp")
                    nc.tensor.transpose(htps[:], helu[:], ident16[:])
                    nc.scalar.activation(
                        ht[:, mac * 128:(mac + 1) * 128], htps[:],
                        mybir.ActivationFunctionType.Copy)

        tc.strict_bb_all_engine_barrier()

        # ---------------- phase C: q / tab2-local + AllGather ----------------
        with tc.tile_pool(name="pc_ps", bufs=4, space="PSUM") as pcps, \
             tc.tile_pool(name="pc_ps2", bufs=2, space="PSUM") as pcps2, \
             tc.tile_pool(name="pc_st", bufs=3) as pcst:
            CH2 = 8
            for sc0 in range(0, NMACRO, CH2):
                nch = min(CH2, NMACRO - sc0)
                st2 = pcst.tile([128, CH2 * ROW2], FP16, tag="st2")
                for k in range(nch):
                    scc = sc0 + k
                    qps = pcps.tile([128, ROW2], FP32, space="PSUM", tag="q")
                    nc.tensor.matmul(qps[:], ht[:, scc * 128:(scc + 1) * 128],
                                     rhs2[:], start=True, stop=True)
                    nc.scalar.activation(
                        st2[:, k * ROW2:(k + 1) * ROW2], qps[:],
                        mybir.ActivationFunctionType.Copy)
                st2v = st2[:].rearrange("p (t c) -> p t c", c=ROW2)
                dst2 = tb2l_d.ap().rearrange("(p t) c -> p t c", t=NMACRO)
                nc.sync.dma_start(
                    dst2[:, sc0:sc0 + nch, :], st2v[:, 0:nch, :])
                nc.vector.tensor_copy(adsl2[:, sc0:sc0 + nch].unsqueeze(2),
                                      st2v[:, 0:nch, 33:34])
            # expand layer-2 window ads
            for w in range(4):
                adw2_ps = pcps2.tile([128, NMACRO], FP32, space="PSUM",
                                     tag="adw2")
                nc.tensor.matmul(adw2_ps[:],
                                 selw_sb[:, 512 + w * 128:512 + (w + 1) * 128],
                                 adsl2[:], start=True, stop=True)
                nc.scalar.activation(
                    adsl2w[:, w * NMACRO:(w + 1) * NMACRO], adw2_ps[:],
                    mybir.ActivationFunctionType.Copy)

        tc.strict_bb_all_engine_barrier()
        with tc.tile_critical():
            nc.gpsimd.collective_compute(
                "AllGather", mybir.AluOpType.bypass,
                replica_groups=[list(range(NCORE))],
                ins=[tb2l_d.ap().opt()],
                outs=[tb2g_d.ap().opt()],
            ).then_inc(cc_sem)
            nc.gpsimd.wait_ge(cc_sem, 1)
        tc.strict_bb_all_engine_barrier()
        # reshape dense AllGather output into 256B-stride gather layout
        # (row g -> g XOR 32768)
        nc.sync.dma_start(tab2_d.ap()[32768:65536, 0:ROW2],
                          tb2g_d.ap()[0:32768, :])
        nc.sync.dma_start(tab2_d.ap()[0:NCORE * CS - 32768, 0:ROW2],
                          tb2g_d.ap()[32768:NCORE * CS, :])
        tc.strict_bb_all_engine_barrier()

        # ---------------- phase D: layer-2 edges ----------------
        gsrc2 = tab2_d.ap()[32768:65536, 0:ROW2S]
        opool = ctx.enter_context(tc.tile_pool(name="oacc", bufs=1))
        oacc = opool.tile([128, NMACRO * C2], FP32)
        with tc.tile_pool(name="ix2", bufs=2) as ixp2, \
             tc.tile_pool(name="pt2", bufs=2) as ptp2, \
             tc.tile_pool(name="g2", bufs=2) as g2p, \
             tc.tile_pool(name="zu2", bufs=2) as zup2, \
             tc.tile_pool(name="msg2", bufs=2) as msgp2, \
             tc.tile_pool(name="adx2_ps", bufs=2, space="PSUM") as adxp2, \
             tc.tile_pool(name="agg2", bufs=4, space="PSUM") as aggp2, \
             tc.tile_pool(name="o2", bufs=2) as o2p:
            for gi, (t0, t1, m0, m1) in enumerate(groups):
                gt = t1 - t0
                o0 = (t0 + gi) * 8
                ni = (gt + 1) * 128
                ixs = ixp2.tile([32, GB * 8], I16, tag="ixs2")
                nc.sync.dma_start(ixs[:, 0:(gt + 1) * 8],
                                  is2_d.ap()[:, o0:o0 + (gt + 1) * 8])
                patg = ptp2.tile([128, GT_MAX * WIN], FP16, tag="patg2")
                nc.sync.dma_start(patg[:, 0:gt * WIN],
                                  pat_d.ap()[:, t0 * WIN:t1 * WIN])
                patTg = ptp2.tile([32, GT_MAX * 128], FP16, tag="patTg2")
                nc.sync.dma_start(patTg[:, 0:gt * 128],
                                  patT_d.ap()[:, t0 * 128:t1 * 128])
                g2 = g2p.tile([128, GB * ROW2S], FP16, tag="g2")
                for s0 in range(0, ni, SUB):
                    n = min(SUB, ni - s0)
                    dma_gather_raw(
                        nc.gpsimd,
                        g2[:, (s0 // 128) * ROW2S:((s0 + n) // 128) * ROW2S]
                        .rearrange("p (j c) -> p j c", c=ROW2S),
                        gsrc2, ixs[:, s0 // 16:(s0 + n) // 16], n, ROW2S,
                        R2STEP, queue_num=qrot())
                adx2 = adxp2.tile([128, GT_MAX], FP32, space="PSUM", tag="adx2")
                for t in range(t0, t1):
                    mac, w, _, _ = sched[t]
                    nc.tensor.matmul(
                        adx2[:, (t - t0):(t - t0 + 1)],
                        patTg[:, (t - t0) * 128:(t - t0 + 1) * 128],
                        adsl2w[0:32, w * NMACRO + mac:w * NMACRO + mac + 1],
                        start=True, stop=True)
                g2v = g2[:].rearrange("p (t c) -> p t c", c=ROW2S)
                z2 = zup2.tile([128, GT_MAX], FP32, tag="z2")
                nc.vector.tensor_tensor(
                    out=z2[:, 0:gt].unsqueeze(2),
                    in0=g2v[:, 0:gt, 32:33],
                    in1=adx2[:, 0:gt].unsqueeze(2),
                    op=mybir.AluOpType.add)
                zs2 = zup2.tile([128, GT_MAX], FP32, tag="zs2")
                nc.vector.tensor_scalar_mul(zs2[:, 0:gt], z2[:, 0:gt], 0.2)
                nc.vector.tensor_tensor(out=z2[:, 0:gt], in0=z2[:, 0:gt],
                                        in1=zs2[:, 0:gt], op=mybir.AluOpType.max)
                u2 = zup2.tile([128, GT_MAX * 2], FP16, tag="u2")
                u2v = u2[:].rearrange("p (t j) -> p t j", j=2)
                nc.scalar.activation(u2v[:, 0:gt, 0:1], z2[:, 0:gt].unsqueeze(2),
                                     mybir.ActivationFunctionType.Exp, bias=nsh2[:])
                nc.vector.tensor_copy(u2v[:, 0:gt, 1:2], u2v[:, 0:gt, 0:1])
                msg2 = msgp2.tile([128, GT_MAX * 34], FP16, tag="m2")
                m2v = msg2[:].rearrange("p (t c) -> p t c", c=34)
                nc.vector.tensor_tensor(
                    out=m2v[:, 0:gt, 0:32].rearrange("p t (c j) -> p t c j", j=2),
                    in0=g2v[:, 0:gt, 0:32].rearrange("p t (c j) -> p t c j", j=2),
                    in1=u2v[:, 0:gt, :].unsqueeze(2).broadcast_to([128, gt, 16, 2]),
                    op=mybir.AluOpType.mult)
                nc.vector.tensor_copy(m2v[:, 0:gt, 32:34], u2v[:, 0:gt, :])
                for mac in range(m0, m1):
                    agg2 = aggp2.tile([128, F2], FP32, space="PSUM", tag="ag2")
                    for t in range(mstart[mac], mstart[mac + 1]):
                        _, w, fw, lw = sched[t]
                        nc.tensor.matmul(
                            agg2[w * WIN:(w + 1) * WIN, :],
                            patg[:, (t - t0) * WIN:(t - t0 + 1) * WIN],
                            msg2[:, (t - t0) * 34:(t - t0) * 34 + F2],
                            start=fw, stop=lw, tile_position=(0, w * WIN))
                    r2 = o2p.tile([128, 1], FP32, tag="r2")
                    nc.vector.reciprocal(r2[:], agg2[:, C2:C2 + 1])
                    nc.vector.tensor_tensor(
                        out=oacc[:, mac * C2:(mac + 1) * C2], in0=agg2[:, 0:C2],
                        in1=r2[:].broadcast_to([128, C2]),
                        op=mybir.AluOpType.mult)
            nc.sync.dma_start(
                out2_d.ap().rearrange("(m p) c -> p m c", p=128),
                oacc[:].rearrange("p (m c) -> p m c", c=C2))

    nc.compile()
    return nc


_CACHE = {}


def run(inputs, trace=False):
    sched, per_core = host_prep(inputs)
    in_maps = make_in_maps(inputs, sched, per_core)
    key = len(sched)
    if key not in _CACHE:
        _CACHE[key] = build_program(sched)
    nc = _CACHE[key]
    res = run_bass_kernel_spmd(nc, in_maps, core_ids=list(range(NCORE)), trace=trace)
    outs = [r["out2"] for r in res.results]
    out = np.zeros((N, C2), np.float32)
    for c in range(NCORE):
        lo = c * CS
        hi = min(lo + CS, N)
        out[lo:hi] = outs[c][: hi - lo]
    return out, res


def kernel(**inputs):
    """Full-input GAT kernel: shards across 8 NeuronCores internally."""
    import numpy as _np
    out, _res = run(inputs)
    return out.astype(_np.float32)
